# revision 1
# baseline (speedup 1.0000x reference)
import numpy as np

NTOT, MD, CD = 1024, 9, 64
P3D, P2D, P1D = 16, 4, 2
NCORES = 8
NLOC = NTOT // NCORES
FT = 512

PAIRS = [(a, a + d) for d in range(MD) for a in range(MD - d)]
TRIPS = [(a, a + d, a + d + e) for e in range(MD) for d in range(MD - e)
         for a in range(MD - d - e)]
N2, N3 = len(PAIRS), len(TRIPS)
DOFF = [0]
for d in range(MD):
    DOFF.append(DOFF[-1] + (MD - d))
KMON = MD + N2 + N3
MOUT = MD * (P3D + P2D + P1D)
MSPLIT = 128
M3SPLIT = MSPLIT - MD - N2

KPERM = list(range(54, 128)) + list(range(0, 9)) + list(range(9, 54))
XR = 74
M2R = 83

W9 = MD * NLOC
_PK_ITEMS = (("uclo", MOUT), ("uchi", MOUT), ("glo", MD), ("ghi", MD),
             ("w1t", 3 * CD), ("wct", 3 * CD))
PK_OFF = {}
_c = 0
for _nm, _w in _PK_ITEMS:
    PK_OFF[_nm] = _c
    _c += _w
PK_BASE = _c

_PROGRAM = {}


def _sym_compress(U3, U2):
    tidx = {t: k for k, t in enumerate(TRIPS)}
    qidx = {q: k for k, q in enumerate(PAIRS)}
    U3c = np.zeros((MD, N3, P3D), np.float64)
    for a in range(MD):
        for b in range(MD):
            for i in range(MD):
                U3c[:, tidx[tuple(sorted((a, b, i)))], :] += U3[:, a, b, i, :]
    U2c = np.zeros((MD, N2, P2D), np.float64)
    for a in range(MD):
        for i in range(MD):
            U2c[:, qidx[tuple(sorted((a, i)))], :] += U2[:, a, i, :]
    return U3c.astype(np.float32), U2c.astype(np.float32)


def _build_ucomb_g(U3c, U2c, U1):
    U = np.zeros((KMON, MOUT), np.float32)
    G = np.zeros((MOUT, MD), np.float32)
    for o in range(MD):
        U[54:54 + N3, o * P3D:(o + 1) * P3D] = U3c[o]
        U[9:9 + N2, 144 + o * P2D:144 + (o + 1) * P2D] = U2c[o]
        U[0:MD, 180 + o * P1D:180 + (o + 1) * P1D] = U1[o]
        G[o * P3D:(o + 1) * P3D, o] = 1.0
        G[144 + o * P2D:144 + (o + 1) * P2D, o] = 1.0
        G[180 + o * P1D:180 + (o + 1) * P1D, o] = 1.0
    return U, G


def _build_program(nloc, repeat=1, stage=4):
    import concourse.bacc as bacc
    import concourse.bass as bass
    from concourse import mybir
    from concourse.tile import TileContext

    f16 = mybir.dt.float16
    f32 = mybir.dt.float32
    AF = mybir.ActivationFunctionType
    F = nloc * CD
    nft = max(1, F // FT)
    ft = min(FT, F)
    w9 = MD * nloc
    lblk = [(0, nloc), (nloc, 4 * nloc), (4 * nloc, 9 * nloc)]
    pkw_in = PK_BASE + w9
    pkw = PK_BASE + 3 * w9

    nc = bacc.Bacc("TRN2", debug=False, enable_asserts=False,
                   num_devices=NCORES, num_swdge_queues=4)

    pk_d = nc.dram_tensor("pk", [128, pkw_in], f16, kind="ExternalInput").ap()
    b12_d = nc.dram_tensor("b12", [CD, 2], f32, kind="ExternalInput").ap()
    wg_d = nc.dram_tensor("wg", [P3D + P2D + P1D, F], f16,
                          kind="ExternalInput").ap()
    out_d = nc.dram_tensor("out", [CD, w9], f32, kind="ExternalOutput").ap()

    with TileContext(nc) as tc:
        with (
            tc.tile_pool(name="const", bufs=1) as const,
            tc.tile_pool(name="big", bufs=1) as big,
            tc.tile_pool(name="work", bufs=2) as work,
            tc.tile_pool(name="ps_a", bufs=1, space="PSUM") as ps_a,
            tc.tile_pool(name="ps_d", bufs=2, space="PSUM") as ps_d,
            tc.tile_pool(name="ps_t", bufs=2, space="PSUM") as ps_t,
        ):
          _dmaq = [0]

          def dma(out, in_, small=False):
              if small:
                  nc.gpsimd.dma_start(out=out, in_=in_)
                  return
              i = _dmaq[0] % 2
              _dmaq[0] += 1
              if i == 0:
                  nc.sync.dma_start(out=out, in_=in_)
              else:
                  nc.scalar.dma_start(out=out, in_=in_)

          def _emit():
            ox = PK_BASE
            oy = PK_BASE + w9
            ot = PK_BASE + 2 * w9
            pk = const.tile([128, pkw], f16)
            sb_uclo = pk[0:128, PK_OFF["uclo"]:PK_OFF["uclo"] + MOUT]
            sb_uchi = pk[0:KMON - 128, PK_OFF["uchi"]:PK_OFF["uchi"] + MOUT]
            sb_glo = pk[0:128, PK_OFF["glo"]:PK_OFF["glo"] + MD]
            sb_ghi = pk[0:MOUT - 128, PK_OFF["ghi"]:PK_OFF["ghi"] + MD]
            sb_w1t = pk[0:CD, PK_OFF["w1t"]:PK_OFF["w1t"] + 3 * CD]
            sb_wct = pk[0:CD, PK_OFF["wct"]:PK_OFF["wct"] + 3 * CD]
            sb_xt = pk[0:CD, ox:ox + w9]
            ysb = pk[0:CD, oy:oy + w9]
            termT = pk[0:CD, ot:ot + w9]
            pk32 = const.tile([CD, 2 + w9], f32)
            sb_b1 = pk32[:, 0:1]
            sb_b2 = pk32[:, 1:2]
            outSB = pk32[:, 2:2 + w9]
            nc.sync.dma_start(out=pk[:, 0:pkw_in], in_=pk_d)
            nc.scalar.dma_start(out=pk32[:, 0:2], in_=b12_d)

            for l, (c0, c1) in enumerate(lblk):
                w_l = sb_w1t[:, l * CD:(l + 1) * CD]
                for s0 in range(c0, c1, FT):
                    s1 = min(s0 + FT, c1)
                    py = ps_a.tile([CD, ft], mybir.dt.float32, name="py",
                                   tag="py", bufs=1)
                    nc.tensor.matmul(py[:, :s1 - s0], lhsT=w_l,
                                     rhs=sb_xt[:, s0:s1], start=True, stop=True)
                    if l == 0:
                        nc.scalar.activation(ysb[:, s0:s1], py[:, :s1 - s0],
                                             AF.Identity, bias=sb_b1)
                    else:
                        nc.scalar.activation(ysb[:, s0:s1], py[:, :s1 - s0],
                                             AF.Copy)

            mon_lo = big.tile([128, F], f16)
            mon_hi = big.tile([KMON - 128, F], f16)

            for l, (c0, c1) in enumerate(lblk):
                m0 = l * l
                for mm in range(2 * l + 1):
                    src = ysb[:, c0 + mm * nloc:c0 + (mm + 1) * nloc]
                    dma(mon_lo[XR + m0 + mm:XR + m0 + mm + 1, :], src,
                        small=True)

            if stage <= 1:
                nc.sync.dma_start(out=out_d, in_=outSB)
                return
            xpre = big.tile([N2, F], f16, tag="shA")
            xsuf = big.tile([N2, F], f16, tag="shB")
            for d in range(MD):
                q0, cnt = DOFF[d], MD - d
                dma(xpre[q0:q0 + cnt, :], mon_lo[XR:XR + cnt, :])
                dma(xsuf[q0:q0 + cnt, :], mon_lo[XR + d:XR + MD, :])
            m2t = big.tile([N2, F], f16, tag="shT")
            nc.vector.tensor_mul(m2t[:], xpre[:], xsuf[:])
            dma(mon_lo[M2R:M2R + N2, :], m2t[:])

            m2rep_a = big.tile([M3SPLIT, F], f16, tag="shA")
            m2rep_b = big.tile([N3 - M3SPLIT, F], f16, tag="shB")
            xrep_a = big.tile([M3SPLIT, F], f16, tag="shC")
            xrep_b = big.tile([N3 - M3SPLIT, F], f16, tag="shD")

            def run_copy(dst_a, dst_b, t0, src, s0, ln):
                if t0 < M3SPLIT:
                    n_lo = min(ln, M3SPLIT - t0)
                    dma(dst_a[t0:t0 + n_lo, :], src[s0:s0 + n_lo, :])
                    if n_lo < ln:
                        dma(dst_b[0:ln - n_lo, :], src[s0 + n_lo:s0 + ln, :])
                else:
                    dma(dst_b[t0 - M3SPLIT:t0 - M3SPLIT + ln, :],
                        src[s0:s0 + ln, :])

            t0 = N2
            run_copy(m2rep_a, m2rep_b, 0, m2t, 0, N2)
            for e in range(1, MD):
                for d in range(MD - e):
                    ln = MD - d - e
                    run_copy(m2rep_a, m2rep_b, t0, m2t, DOFF[d], ln)
                    t0 += ln
            t0 = 0
            for e in range(MD):
                ln = N2 - DOFF[e]
                run_copy(xrep_a, xrep_b, t0, xsuf, DOFF[e], ln)
                t0 += ln
            nc.vector.tensor_mul(mon_lo[0:M3SPLIT, :], m2rep_a[:], xrep_a[:])
            nc.vector.tensor_mul(mon_hi[:], m2rep_b[:], xrep_b[:])

            if stage <= 2:
                nc.sync.dma_start(out=out_d, in_=outSB)
                return
            wrep_lo = big.tile([128, F], f16, tag="shC")
            wrep_hi = big.tile([MOUT - 128, F], f16, tag="shD")
            src8 = bass.AP(tensor=wg_d.tensor, offset=0,
                           ap=[[0, 8], [F, 16], [1, F]])
            nc.sync.dma_start(out=wrep_lo[:], in_=src8)
            nc.scalar.dma_start(out=wrep_hi[0:16, :], in_=wg_d[0:16, :])
            src2 = bass.AP(tensor=wg_d.tensor, offset=16 * F,
                           ap=[[0, MD], [F, 4], [1, F]])
            nc.sync.dma_start(out=wrep_hi[16:52, :], in_=src2)
            src1 = bass.AP(tensor=wg_d.tensor, offset=20 * F,
                           ap=[[0, MD], [F, 2], [1, F]])
            nc.scalar.dma_start(out=wrep_hi[52:70, :], in_=src1)

            if stage <= 3:
                nc.sync.dma_start(out=out_d, in_=outSB)
                return
            termSB = big.tile([MD, F], f16, tag="shT")
            for j in range(nft):
                js = slice(j * ft, (j + 1) * ft)
                dlo = ps_d.tile([128, ft], mybir.dt.float32, name="dlo", tag="dlo")
                dhi = ps_d.tile([MOUT - 128, ft], mybir.dt.float32, name="dhi",
                                tag="dhi")
                nc.tensor.matmul(dlo[:], lhsT=sb_uclo[:, 0:128],
                                 rhs=mon_lo[:, js], start=True, stop=False)
                nc.tensor.matmul(dlo[:], lhsT=sb_uchi[:, 0:128],
                                 rhs=mon_hi[:, js], start=False, stop=True)
                nc.tensor.matmul(dhi[:], lhsT=sb_uclo[:, 128:MOUT],
                                 rhs=mon_lo[:, js], start=True, stop=False)
                nc.tensor.matmul(dhi[:], lhsT=sb_uchi[:, 128:MOUT],
                                 rhs=mon_hi[:, js], start=False, stop=True)
                dw_lo = work.tile([128, ft], f16, name="dw_lo", tag="dw_lo")
                dw_hi = work.tile([MOUT - 128, ft], f16, name="dw_hi", tag="dw_hi")
                nc.vector.tensor_mul(dw_lo[:], dlo[:], wrep_lo[:, js])
                nc.vector.tensor_mul(dw_hi[:], dhi[:], wrep_hi[:, js])
                pt = ps_t.tile([MD, ft], mybir.dt.float32, name="pt", tag="pt")
                nc.tensor.matmul(pt[:], lhsT=sb_glo, rhs=dw_lo[:],
                                 start=True, stop=False)
                nc.tensor.matmul(pt[:], lhsT=sb_ghi, rhs=dw_hi[:],
                                 start=False, stop=True)
                nc.scalar.activation(termSB[:, js], pt[:], AF.Copy)

            for o in range(MD):
                dma(termT[:, o * nloc:(o + 1) * nloc], termSB[o:o + 1, :],
                    small=True)

            for l, (c0, c1) in enumerate(lblk):
                w_l = sb_wct[:, l * CD:(l + 1) * CD]
                for s0 in range(c0, c1, FT):
                    s1 = min(s0 + FT, c1)
                    pf = ps_a.tile([CD, ft], mybir.dt.float32, name="pf",
                                   tag="pf", bufs=1)
                    nc.tensor.matmul(pf[:, :s1 - s0], lhsT=w_l,
                                     rhs=termT[:, s0:s1], start=True, stop=True)
                    if l == 0:
                        nc.scalar.activation(outSB[:, s0:s1], pf[:, :s1 - s0],
                                             AF.Identity, bias=sb_b2)
                    else:
                        nc.scalar.activation(outSB[:, s0:s1], pf[:, :s1 - s0],
                                             AF.Copy)
            nc.sync.dma_start(out=out_d, in_=outSB)

          if repeat > 1:
              with tc.For_i(0, repeat, 1):
                  _emit()
          else:
              _emit()

    return nc


def _get_program(nloc, repeat=1, stage=4):
    key = (nloc, repeat, stage)
    if key not in _PROGRAM:
        nc = _build_program(nloc, repeat, stage)
        nc.compile()
        _PROGRAM[key] = nc
    return _PROGRAM[key]


def make_in_maps(irreps_x, atomic_numbers, w_fc1, b_fc1, U3, W3, U2, W2, U1, W1,
                 w_lin, w_fc2, b_fc2, nloc=NLOC, ncores=NCORES):
    irreps_x = np.asarray(irreps_x, np.float32)
    a_n = np.asarray(atomic_numbers).astype(np.int64)
    U3c, U2c = _sym_compress(np.asarray(U3, np.float64),
                             np.asarray(U2, np.float64))
    Ucomb, G = _build_ucomb_g(U3c, U2c, np.asarray(U1, np.float32))
    w_comb = np.einsum('lde,lec->ldc', np.asarray(w_fc2, np.float32),
                       np.asarray(w_lin, np.float32))
    w1t = np.concatenate([np.asarray(w_fc1, np.float32)[l].T for l in range(3)],
                         axis=1)
    wct = np.concatenate([w_comb[l].T for l in range(3)], axis=1)
    w3g = np.asarray(W3, np.float32)[a_n]
    w2g = np.asarray(W2, np.float32)[a_n]
    w1g = np.asarray(W1, np.float32)[a_n]
    F = nloc * CD
    w9 = MD * nloc
    pkw_in = PK_BASE + w9

    def put(buf, nm, arr):
        o = PK_OFF[nm]
        arr = np.asarray(arr, np.float32).astype(np.float16)
        buf[:arr.shape[0], o:o + arr.shape[1]] = arr

    uc_p = Ucomb[KPERM]
    b12 = np.stack([np.asarray(b_fc1, np.float32),
                    np.asarray(b_fc2, np.float32)], axis=1).astype(np.float32)
    in_maps = []
    for core in range(ncores):
        s = slice(core * nloc, (core + 1) * nloc)
        parts = []
        for l in range(3):
            seg = irreps_x[s, l * l:(l + 1) * (l + 1), :]
            parts.append(seg.transpose(2, 1, 0).reshape(CD, -1))
        xt = np.concatenate(parts, axis=1)
        pk = np.zeros((128, pkw_in), np.float16)
        put(pk, "uclo", uc_p[0:128])
        put(pk, "uchi", Ucomb[128:KMON])
        put(pk, "glo", G[0:128])
        put(pk, "ghi", G[128:MOUT])
        put(pk, "w1t", w1t)
        put(pk, "wct", wct)
        pk[:CD, PK_BASE:PK_BASE + w9] = xt.astype(np.float16)
        wg = np.concatenate([
            w3g[s].transpose(1, 2, 0).reshape(P3D, F),
            w2g[s].transpose(1, 2, 0).reshape(P2D, F),
            w1g[s].transpose(1, 2, 0).reshape(P1D, F),
        ], axis=0)
        in_maps.append({
            "pk": pk,
            "b12": b12,
            "wg": wg.astype(np.float16),
        })
    return in_maps


def unpack_out(o, nloc=NLOC):
    return np.ascontiguousarray(
        o.reshape(CD, MD, nloc).transpose(2, 1, 0)).astype(np.float32)


def kernel(**inputs):
    from concourse import bass_utils
    in_maps = make_in_maps(**inputs)
    nc = _get_program(NLOC)
    res = bass_utils.run_bass_kernel_spmd(nc, in_maps,
                                          core_ids=list(range(NCORES)))
    outs = [unpack_out(res.results[c]["out"]) for c in range(NCORES)]
    return np.concatenate(outs, axis=0).astype(np.float32)



# revision 13
# speedup vs baseline: 1.3380x; 1.3380x over previous
import numpy as np

NTOT, MD, CD = 1024, 9, 64
P3D, P2D, P1D = 16, 4, 2
NCORES = 8
NLOC = NTOT // NCORES
FT = 512

PAIRS = [(a, a + d) for d in range(MD) for a in range(MD - d)]
TRIPS = [(a, a + d, a + d + e) for e in range(MD) for d in range(MD - e)
         for a in range(MD - d - e)]
N2, N3 = len(PAIRS), len(TRIPS)
QIDX = {q: k for k, q in enumerate(PAIRS)}
DOFF = [0]
for _d in range(MD):
    DOFF.append(DOFF[-1] + (MD - _d))
E0 = N2
E1 = N3 - E0
KMON = MD + N2 + N3
MOUT = MD * (P3D + P2D + P1D)
KLO = E1
KHI = 109

_PK_ITEMS = (("uclo", MOUT), ("uchi", MOUT), ("glo", MD), ("ghi", MD),
             ("w1t", 3 * CD), ("wct", 3 * CD), ("sel1", E1))
PK_OFF = {}
_c = 0
for _nm, _w in _PK_ITEMS:
    PK_OFF[_nm] = _c
    _c += _w
PK_BASE = _c
W9 = MD * NLOC

_PROGRAM = {}


def _sym_compress(U3, U2):
    tidx = {t: k for k, t in enumerate(TRIPS)}
    U3c = np.zeros((MD, N3, P3D), np.float64)
    for a in range(MD):
        for b in range(MD):
            for i in range(MD):
                U3c[:, tidx[tuple(sorted((a, b, i)))], :] += U3[:, a, b, i, :]
    U2c = np.zeros((MD, N2, P2D), np.float64)
    for a in range(MD):
        for i in range(MD):
            U2c[:, QIDX[tuple(sorted((a, i)))], :] += U2[:, a, i, :]
    return U3c.astype(np.float32), U2c.astype(np.float32)


def _build_ucomb_g(U3c, U2c, U1):
    U = np.zeros((KMON, MOUT), np.float32)
    G = np.zeros((MOUT, MD), np.float32)
    for o in range(MD):
        U[MD + N2:, o * P3D:(o + 1) * P3D] = U3c[o]
        U[MD:MD + N2, 144 + o * P2D:144 + (o + 1) * P2D] = U2c[o]
        U[0:MD, 180 + o * P1D:180 + (o + 1) * P1D] = U1[o]
        G[o * P3D:(o + 1) * P3D, o] = 1.0
        G[144 + o * P2D:144 + (o + 1) * P2D, o] = 1.0
        G[180 + o * P1D:180 + (o + 1) * P1D, o] = 1.0
    UA = U[MD + N2 + E0:]
    UB = np.zeros((KHI, MOUT), np.float32)
    UB[0:N2] = U[MD:MD + N2]
    UB[N2:N2 + MD] = U[0:MD]
    UB[64:64 + E0] = U[MD + N2:MD + N2 + E0]
    return UA, UB, G


def _build_sel1():
    sel1 = np.zeros((N2, E1), np.float32)
    for t, (a, b, i) in enumerate(TRIPS[E0:]):
        sel1[QIDX[(a, b)], t] = 1.0
    return sel1


def _build_program(nloc, repeat=1, stage=4):
    import concourse.bacc as bacc
    from concourse import mybir
    from concourse.tile import TileContext

    f16 = mybir.dt.float16
    f32 = mybir.dt.float32
    AF = mybir.ActivationFunctionType
    F = nloc * CD
    nft = F // FT
    w9 = MD * nloc
    lblk = [(0, nloc), (nloc, 4 * nloc), (4 * nloc, 9 * nloc)]
    pkw = PK_BASE + w9

    nc = bacc.Bacc("TRN2", debug=False, enable_asserts=False,
                   num_devices=NCORES, num_swdge_queues=4)

    pk_d = nc.dram_tensor("pk", [128, pkw], f16, kind="ExternalInput").ap()
    b12_d = nc.dram_tensor("b12", [CD, 2], f32, kind="ExternalInput").ap()
    wrep_d = nc.dram_tensor("wrep", [MOUT, F], f16, kind="ExternalInput").ap()
    out_d = nc.dram_tensor("out", [CD, w9], f32, kind="ExternalOutput").ap()

    with TileContext(nc) as tc:
        with (
            tc.tile_pool(name="const", bufs=1) as const,
            tc.tile_pool(name="big", bufs=1) as big,
            tc.tile_pool(name="work", bufs=2) as work,
            tc.tile_pool(name="ps_r", bufs=2, space="PSUM") as ps_r,
            tc.tile_pool(name="ps_d", bufs=2, space="PSUM") as ps_d,
            tc.tile_pool(name="ps_t", bufs=2, space="PSUM") as ps_t,
        ):
          pk = const.tile([128, pkw], f16)
          pk32 = const.tile([CD, 2], f32)

          def pks(nm, r0, r1):
              return pk[r0:r1, PK_OFF[nm]:PK_OFF[nm] + dict(_PK_ITEMS)[nm]]

          uclo = pks("uclo", 0, KLO)
          uchi = pks("uchi", 0, KHI)
          glo = pks("glo", 0, 128)
          ghi = pks("ghi", 0, MOUT - 128)
          w1t = pks("w1t", 0, CD)
          wct = pks("wct", 0, CD)
          sel1 = pks("sel1", 0, N2)
          xt = pk[0:CD, PK_BASE:PK_BASE + w9]
          sb_b1 = pk32[:, 0:1]
          sb_b2 = pk32[:, 1:2]

          ysb = big.tile([CD, w9], f16)
          xsb = big.tile([MD, F], f16)
          xpre = big.tile([N2, F], f16)
          xsuf = big.tile([N2, F], f16)
          xrep = big.tile([E1, F], f16)
          mon_lo = big.tile([KLO, F], f16)
          mon_hi = big.tile([KHI, F], f16)
          wrep_lo = big.tile([128, F], f16)
          wrep_hi = big.tile([MOUT - 128, F], f16)
          termSB = big.tile([MD, F], f16)
          termT = big.tile([CD, w9], f16)
          outSB = big.tile([CD, w9], f32)

          nc.gpsimd.memset(mon_hi[32:64, :], 0.0)

          def _emit():
            nc.sync.dma_start(out=pk[:, :], in_=pk_d)
            nc.scalar.dma_start(out=pk32[:, :], in_=b12_d)

            for l, (c0, c1) in enumerate(lblk):
                w_l = w1t[:, l * CD:(l + 1) * CD]
                for s0 in range(c0, c1, FT):
                    s1 = min(s0 + FT, c1)
                    py = ps_t.tile([CD, FT], f32, name="py", tag="t")
                    nc.tensor.matmul(py[:, :s1 - s0], lhsT=w_l,
                                     rhs=xt[:, s0:s1], start=True, stop=True)
                    if l == 0:
                        nc.scalar.activation(ysb[:, s0:s1], py[:, :s1 - s0],
                                             AF.Identity, bias=sb_b1)
                    else:
                        nc.scalar.activation(ysb[:, s0:s1], py[:, :s1 - s0],
                                             AF.Copy)

            for m in range(MD):
                nc.gpsimd.dma_start(out=xsb[m:m + 1, :],
                                    in_=ysb[:, m * nloc:(m + 1) * nloc])
            nc.scalar.dma_start(out=mon_hi[N2:N2 + MD, :], in_=xsb[:, :])

            for d in range(MD):
                q0, cnt = DOFF[d], MD - d
                nc.sync.dma_start(out=xpre[q0:q0 + cnt, :],
                                  in_=xsb[0:cnt, :])
                nc.scalar.dma_start(out=xsuf[q0:q0 + cnt, :],
                                    in_=xsb[d:MD, :])
            t0 = 0
            for e in range(1, MD):
                ln = N2 - DOFF[e]
                nc.gpsimd.dma_start(out=xrep[t0:t0 + ln, :],
                                    in_=xsuf[DOFF[e]:N2, :])
                t0 += ln

            nc.vector.tensor_mul(mon_hi[0:N2, :], xpre[:, :], xsuf[:, :])

            if stage >= 3:
                h = F // 2
                nc.sync.dma_start(out=wrep_lo[:, 0:h], in_=wrep_d[0:128, 0:h])
                nc.sync.dma_start(out=wrep_lo[:, h:F], in_=wrep_d[0:128, h:F])
                nc.scalar.dma_start(out=wrep_hi[:, 0:h],
                                    in_=wrep_d[128:MOUT, 0:h])
                nc.scalar.dma_start(out=wrep_hi[:, h:F],
                                    in_=wrep_d[128:MOUT, h:F])

            if stage <= 1:
                nc.sync.dma_start(out=out_d, in_=outSB)
                return

            for j in range(nft):
                js = slice(j * FT, (j + 1) * FT)
                nc.gpsimd.tensor_mul(mon_hi[64:64 + E0, js],
                                     mon_hi[0:N2, js], xsuf[:, js])
                ps1 = ps_r.tile([E1, FT], f32, name="ps1", tag="r1")
                nc.tensor.matmul(ps1[:], lhsT=sel1, rhs=mon_hi[0:N2, js],
                                 start=True, stop=True)
                nc.vector.tensor_mul(mon_lo[:, js], xrep[:, js], ps1[:])

                if stage <= 2:
                    continue
                dlo = ps_d.tile([128, FT], f32, name="dlo", tag="dlo")
                dhi = ps_d.tile([MOUT - 128, FT], f32, name="dhi", tag="dhi")
                nc.tensor.matmul(dlo[:], lhsT=uclo[:, 0:128],
                                 rhs=mon_lo[:, js], start=True, stop=False)
                nc.tensor.matmul(dlo[:], lhsT=uchi[:, 0:128],
                                 rhs=mon_hi[0:KHI, js], start=False, stop=True)
                nc.tensor.matmul(dhi[:], lhsT=uclo[:, 128:MOUT],
                                 rhs=mon_lo[:, js], start=True, stop=False)
                nc.tensor.matmul(dhi[:], lhsT=uchi[:, 128:MOUT],
                                 rhs=mon_hi[0:KHI, js], start=False, stop=True)
                dsl = work.tile([128, FT], f16, name="dsl", tag="dsl")
                dsh = work.tile([MOUT - 128, FT], f16, name="dsh", tag="dsh")
                nc.scalar.activation(dsl[:], dlo[:], AF.Copy)
                nc.scalar.activation(dsh[:], dhi[:], AF.Copy)
                dwl = work.tile([128, FT], f16, name="dwl", tag="dwl")
                dwh = work.tile([MOUT - 128, FT], f16, name="dwh", tag="dwh")
                nc.vector.tensor_mul(dwl[:], dsl[:], wrep_lo[:, js])
                nc.vector.tensor_mul(dwh[:], dsh[:], wrep_hi[:, js])
                pt = ps_t.tile([MD, FT], f32, name="pt", tag="t")
                nc.tensor.matmul(pt[:], lhsT=glo, rhs=dwl[:],
                                 start=True, stop=False)
                nc.tensor.matmul(pt[:], lhsT=ghi, rhs=dwh[:],
                                 start=False, stop=True)
                nc.scalar.activation(termSB[:, js], pt[:], AF.Copy)

            if stage <= 2:
                nc.sync.dma_start(out=out_d, in_=outSB)
                return

            for o in range(MD):
                dst = termT[:, o * nloc:(o + 1) * nloc]
                src = termSB[o:o + 1, :]
                if o % 2 == 0:
                    nc.sync.dma_start(out=dst, in_=src)
                else:
                    nc.gpsimd.dma_start(out=dst, in_=src)

            for l, (c0, c1) in enumerate(lblk):
                w_l = wct[:, l * CD:(l + 1) * CD]
                for s0 in range(c0, c1, FT):
                    s1 = min(s0 + FT, c1)
                    pf = ps_t.tile([CD, FT], f32, name="pf", tag="t")
                    nc.tensor.matmul(pf[:, :s1 - s0], lhsT=w_l,
                                     rhs=termT[:, s0:s1], start=True, stop=True)
                    if l == 0:
                        nc.scalar.activation(outSB[:, s0:s1], pf[:, :s1 - s0],
                                             AF.Identity, bias=sb_b2)
                    else:
                        nc.scalar.activation(outSB[:, s0:s1], pf[:, :s1 - s0],
                                             AF.Copy)
            nc.sync.dma_start(out=out_d, in_=outSB)

          if repeat > 1:
              with tc.For_i(0, repeat, 1):
                  _emit()
          else:
              _emit()

    return nc


def _get_program(nloc, repeat=1, stage=4):
    key = (nloc, repeat, stage)
    if key not in _PROGRAM:
        nc = _build_program(nloc, repeat, stage)
        nc.compile()
        _PROGRAM[key] = nc
    return _PROGRAM[key]


def make_in_maps(irreps_x, atomic_numbers, w_fc1, b_fc1, U3, W3, U2, W2, U1, W1,
                 w_lin, w_fc2, b_fc2, nloc=NLOC, ncores=NCORES):
    irreps_x = np.asarray(irreps_x, np.float32)
    a_n = np.asarray(atomic_numbers).astype(np.int64)
    U3c, U2c = _sym_compress(np.asarray(U3, np.float64),
                             np.asarray(U2, np.float64))
    UA, UB, G = _build_ucomb_g(U3c, U2c, np.asarray(U1, np.float32))
    sel1 = _build_sel1()
    w_comb = np.einsum('lde,lec->ldc', np.asarray(w_fc2, np.float32),
                       np.asarray(w_lin, np.float32))
    w1t = np.concatenate([np.asarray(w_fc1, np.float32)[l].T for l in range(3)],
                         axis=1)
    wct = np.concatenate([w_comb[l].T for l in range(3)], axis=1)
    w3g = np.asarray(W3, np.float32)[a_n]
    w2g = np.asarray(W2, np.float32)[a_n]
    w1g = np.asarray(W1, np.float32)[a_n]
    F = nloc * CD

    def put(buf, nm, arr, r0=0):
        o = PK_OFF[nm]
        arr = np.asarray(arr, np.float32).astype(np.float16)
        buf[r0:r0 + arr.shape[0], o:o + arr.shape[1]] = arr

    b12 = np.stack([np.asarray(b_fc1, np.float32),
                    np.asarray(b_fc2, np.float32)], axis=1).astype(np.float32)
    in_maps = []
    for core in range(ncores):
        s = slice(core * nloc, (core + 1) * nloc)
        parts = []
        for l in range(3):
            seg = irreps_x[s, l * l:(l + 1) * (l + 1), :]
            parts.append(seg.transpose(2, 1, 0).reshape(CD, -1))
        xtc = np.concatenate(parts, axis=1)
        pk = np.zeros((128, PK_BASE + MD * nloc), np.float16)
        put(pk, "uclo", UA)
        put(pk, "uchi", UB)
        put(pk, "glo", G[0:128])
        put(pk, "ghi", G[128:MOUT])
        put(pk, "w1t", w1t)
        put(pk, "wct", wct)
        put(pk, "sel1", sel1)
        pk[:CD, PK_BASE:PK_BASE + MD * nloc] = xtc.astype(np.float16)
        wg3 = w3g[s].transpose(1, 2, 0).reshape(P3D, F)
        wg2 = w2g[s].transpose(1, 2, 0).reshape(P2D, F)
        wg1 = w1g[s].transpose(1, 2, 0).reshape(P1D, F)
        wrep = np.concatenate([np.tile(wg3, (MD, 1)), np.tile(wg2, (MD, 1)),
                               np.tile(wg1, (MD, 1))], axis=0)
        in_maps.append({
            "pk": pk,
            "b12": b12,
            "wrep": wrep.astype(np.float16),
        })
    return in_maps


def unpack_out(o, nloc=NLOC):
    return np.ascontiguousarray(
        o.reshape(CD, MD, nloc).transpose(2, 1, 0)).astype(np.float32)


def kernel(**inputs):
    from concourse import bass_utils
    in_maps = make_in_maps(**inputs)
    nc = _get_program(NLOC)
    res = bass_utils.run_bass_kernel_spmd(nc, in_maps,
                                          core_ids=list(range(NCORES)))
    outs = [unpack_out(res.results[c]["out"]) for c in range(NCORES)]
    return np.concatenate(outs, axis=0).astype(np.float32)


# revision 27
# speedup vs baseline: 1.7090x; 1.2773x over previous
import numpy as np

NTOT, MD, CD = 1024, 9, 64
P3D, P2D, P1D = 16, 4, 2
NCORES = 8
NLOC = NTOT // NCORES
FT = 512

PAIRS = [(a, a + d) for d in range(MD) for a in range(MD - d)]
TRIPS = [(a, a + d, a + d + e) for e in range(MD) for d in range(MD - e)
         for a in range(MD - d - e)]
N2, N3 = len(PAIRS), len(TRIPS)
QIDX = {q: k for k, q in enumerate(PAIRS)}
DOFF = [0]
for _d in range(MD):
    DOFF.append(DOFF[-1] + (MD - _d))
E0 = N2
E1 = N3 - E0
KMON = MD + N2 + N3
MOUT = MD * (P3D + P2D + P1D)
KLO = E1
KHI = 109

_PK_ITEMS = (("uclo", MOUT), ("uchi", MOUT), ("glo", MD), ("ghi", MD),
             ("w1t", 3 * CD), ("wct", 3 * CD), ("sel1", E1))
PK_OFF = {}
_c = 0
for _nm, _w in _PK_ITEMS:
    PK_OFF[_nm] = _c
    _c += _w
PK_BASE = _c
W9 = MD * NLOC

_PROGRAM = {}


def _sym_compress(U3, U2):
    tidx = {t: k for k, t in enumerate(TRIPS)}
    U3c = np.zeros((MD, N3, P3D), np.float64)
    for a in range(MD):
        for b in range(MD):
            for i in range(MD):
                U3c[:, tidx[tuple(sorted((a, b, i)))], :] += U3[:, a, b, i, :]
    U2c = np.zeros((MD, N2, P2D), np.float64)
    for a in range(MD):
        for i in range(MD):
            U2c[:, QIDX[tuple(sorted((a, i)))], :] += U2[:, a, i, :]
    return U3c.astype(np.float32), U2c.astype(np.float32)


def _build_ucomb_g(U3c, U2c, U1):
    U = np.zeros((KMON, MOUT), np.float32)
    G = np.zeros((MOUT, MD), np.float32)
    for o in range(MD):
        U[MD + N2:, o * P3D:(o + 1) * P3D] = U3c[o]
        U[MD:MD + N2, 144 + o * P2D:144 + (o + 1) * P2D] = U2c[o]
        U[0:MD, 180 + o * P1D:180 + (o + 1) * P1D] = U1[o]
        G[o * P3D:(o + 1) * P3D, o] = 1.0
        G[144 + o * P2D:144 + (o + 1) * P2D, o] = 1.0
        G[180 + o * P1D:180 + (o + 1) * P1D, o] = 1.0
    UA = U[MD + N2 + E0:]
    UB = np.zeros((KHI, MOUT), np.float32)
    UB[0:N2] = U[MD:MD + N2]
    UB[N2:N2 + MD] = U[0:MD]
    UB[64:64 + E0] = U[MD + N2:MD + N2 + E0]
    return UA, UB, G


def _build_sel1():
    sel1 = np.zeros((N2, E1), np.float32)
    for t, (a, b, i) in enumerate(TRIPS[E0:]):
        sel1[QIDX[(a, b)], t] = 1.0
    return sel1


def _build_program(nloc, repeat=1, stage=6):
    import concourse.bacc as bacc
    from concourse import mybir
    from concourse.tile import TileContext

    f16 = mybir.dt.float16
    f32 = mybir.dt.float32
    AF = mybir.ActivationFunctionType
    F = nloc * CD
    nft = F // FT
    w9 = MD * nloc
    lblk = [(0, nloc), (nloc, 4 * nloc), (4 * nloc, 9 * nloc)]
    pkw = PK_BASE + w9

    nc = bacc.Bacc("TRN2", debug=False, enable_asserts=False,
                   num_devices=NCORES, num_swdge_queues=4)

    pk_d = nc.dram_tensor("pk", [128, pkw], f16, kind="ExternalInput").ap()
    b12_d = nc.dram_tensor("b12", [CD, 2], f32, kind="ExternalInput").ap()
    wrep_d = nc.dram_tensor("wrep", [MOUT, F], f16, kind="ExternalInput").ap()
    out_d = nc.dram_tensor("out", [CD, w9], f32, kind="ExternalOutput").ap()
    x_d = nc.dram_tensor("x_sc", [MD, F], f16, kind="Internal").ap()
    xpre_d = nc.dram_tensor("xpre_sc", [N2, F], f16, kind="Internal").ap()
    xsuf_d = nc.dram_tensor("xsuf_sc", [N2, F], f16, kind="Internal").ap()

    FT2 = 2 * FT

    with TileContext(nc) as tc:
        with (
            tc.tile_pool(name="const", bufs=2) as const,
            tc.tile_pool(name="big", bufs=2) as big,
            tc.tile_pool(name="big1", bufs=1) as big1,
            tc.tile_pool(name="work", bufs=2) as work,
            tc.tile_pool(name="ps_r", bufs=1, space="PSUM") as ps_r,
            tc.tile_pool(name="ps_d", bufs=1, space="PSUM") as ps_d,
            tc.tile_pool(name="ps_t", bufs=1, space="PSUM") as ps_t,
            tc.tile_pool(name="ps_f", bufs=1, space="PSUM") as ps_f,
        ):
          def _emit():
            pk = const.tile([128, pkw], f16, name="pk", tag="pk")
            pk32 = const.tile([CD, 2], f32, name="pk32", tag="pk32")

            def pks(nm, r0, r1):
                return pk[r0:r1, PK_OFF[nm]:PK_OFF[nm] + dict(_PK_ITEMS)[nm]]

            uclo = pks("uclo", 0, KLO)
            uchi = pks("uchi", 0, KHI)
            glo = pks("glo", 0, 128)
            ghi = pks("ghi", 0, MOUT - 128)
            w1t = pks("w1t", 0, CD)
            wct = pks("wct", 0, CD)
            sel1 = pks("sel1", 0, N2)
            xt = pk[0:CD, PK_BASE:PK_BASE + w9]
            sb_b1 = pk32[:, 0:1]
            sb_b2 = pk32[:, 1:2]

            ysb = big.tile([CD, w9], f16, name="ysb", tag="ysb")
            xsb = big.tile([MD, F], f16, name="xsb", tag="xsb")
            xpre = big.tile([N2, F], f16, name="xpre", tag="xpre")
            xsuf = big.tile([N2, F], f16, name="xsuf", tag="xsuf")
            xrep = big.tile([E1, F], f16, name="xrep", tag="xrep")
            mon_hi = big.tile([KHI, F], f16, name="mon_hi", tag="mon_hi")
            mon_lo = big1.tile([KLO, F], f16, name="mon_lo", tag="mon_lo")
            wrep_lo = big1.tile([128, F], f16, name="wrep_lo", tag="wrep_lo")
            wrep_hi = big1.tile([MOUT - 128, F], f16, name="wrep_hi",
                                tag="wrep_hi")
            termSB = big1.tile([MD, F], f16, name="termSB", tag="termSB")
            termT = big1.tile([CD, w9], f16, name="termT", tag="termT")
            outSB = big1.tile([CD, w9], f32, name="outSB", tag="outSB")

            nc.sync.dma_start(out=pk[:, :], in_=pk_d)
            nc.scalar.dma_start(out=pk32[:, :], in_=b12_d)
            if stage <= 5:
                nc.gpsimd.memset(outSB[:, :], 0.0)

            for l, (c0, c1) in enumerate(lblk):
                w_l = w1t[:, l * CD:(l + 1) * CD]
                for s0 in range(c0, c1, FT):
                    s1 = min(s0 + FT, c1)
                    py = ps_f.tile([CD, FT], f32, name="py", tag="f")
                    nc.tensor.matmul(py[:, :s1 - s0], lhsT=w_l,
                                     rhs=xt[:, s0:s1], start=True, stop=True)
                    if l == 0:
                        nc.scalar.activation(ysb[:, s0:s1], py[:, :s1 - s0],
                                             AF.Identity, bias=sb_b1)
                    else:
                        nc.scalar.activation(ysb[:, s0:s1], py[:, :s1 - s0],
                                             AF.Copy)

            if stage <= 1:
                nc.sync.dma_start(out=out_d, in_=outSB)
                return

            for m in range(MD):
                eng = (nc.sync, nc.scalar, nc.gpsimd)[m % 3]
                eng.dma_start(out=x_d[m:m + 1, :],
                              in_=ysb[:, m * nloc:(m + 1) * nloc])
            nc.gpsimd.dma_start(out=xsb[:, :], in_=x_d)
            nc.scalar.dma_start(out=mon_hi[N2:N2 + MD, :], in_=x_d)
            nc.gpsimd.dma_start(out=mon_hi[54:63, :], in_=x_d)
            nc.gpsimd.dma_start(out=mon_hi[63:64, :], in_=x_d[0:1, :])

            if stage <= 2:
                nc.sync.dma_start(out=out_d, in_=outSB)
                return

            for d in range(MD):
                q0, cnt = DOFF[d], MD - d
                nc.sync.dma_start(out=xpre_d[q0:q0 + cnt, :],
                                  in_=x_d[0:cnt, :])
                nc.scalar.dma_start(out=xsuf_d[q0:q0 + cnt, :],
                                    in_=x_d[d:MD, :])
            nc.sync.dma_start(out=xpre[:, :], in_=xpre_d)
            nc.scalar.dma_start(out=xsuf[:, :], in_=xsuf_d)
            t0 = 0
            for e in range(1, MD):
                ln = N2 - DOFF[e]
                eng = (nc.sync, nc.scalar)[e % 2]
                eng.dma_start(out=xrep[t0:t0 + ln, :],
                              in_=xsuf_d[DOFF[e]:N2, :])
                t0 += ln

            nc.vector.tensor_mul(mon_hi[0:N2, :], xpre[:, :], xsuf[:, :])
            h2 = F // 2
            nc.vector.tensor_mul(mon_hi[64:64 + E0, 0:h2],
                                 mon_hi[0:N2, 0:h2], xsuf[:, 0:h2])
            nc.vector.tensor_mul(mon_hi[64:64 + E0, h2:F],
                                 mon_hi[0:N2, h2:F], xsuf[:, h2:F])

            if stage <= 3:
                nc.sync.dma_start(out=out_d, in_=outSB)
                return

            if stage >= 5:
                h = F // 2
                nc.sync.dma_start(out=wrep_lo[:, 0:h], in_=wrep_d[0:128, 0:h])
                nc.scalar.dma_start(out=wrep_lo[:, h:F],
                                    in_=wrep_d[0:128, h:F])
                nc.sync.dma_start(out=wrep_hi[:, 0:h],
                                  in_=wrep_d[128:MOUT, 0:h])
                nc.scalar.dma_start(out=wrep_hi[:, h:F],
                                    in_=wrep_d[128:MOUT, h:F])

            for p in range(nft // 2):
                jp = slice(p * FT2, (p + 1) * FT2)
                ps1 = ps_r.tile([E1, FT2], f32, name="ps1", tag="r1")
                for h in range(2):
                    js = slice((2 * p + h) * FT, (2 * p + h + 1) * FT)
                    hs = slice(h * FT, (h + 1) * FT)
                    nc.tensor.matmul(ps1[:, hs], lhsT=sel1,
                                     rhs=mon_hi[0:N2, js],
                                     start=True, stop=True)
                nc.vector.tensor_mul(mon_lo[:, jp], xrep[:, jp], ps1[:])
                if stage <= 4:
                    continue
                dlo = ps_d.tile([128, FT2], f32, name="dlo", tag="dlo")
                pt = ps_t.tile([MD, FT2], f32, name="pt", tag="t")
                for h in range(2):
                    js = slice((2 * p + h) * FT, (2 * p + h + 1) * FT)
                    hs = slice(h * FT, (h + 1) * FT)
                    nc.tensor.matmul(dlo[:, hs], lhsT=uclo[:, 0:128],
                                     rhs=mon_lo[:, js], start=True, stop=False)
                    nc.tensor.matmul(dlo[:, hs], lhsT=uchi[:, 0:128],
                                     rhs=mon_hi[0:KHI, js],
                                     start=False, stop=True)
                dwl = work.tile([128, FT2], f16, name="dwl", tag="dwl")
                nc.vector.tensor_mul(dwl[:], dlo[:], wrep_lo[:, jp])
                for h in range(2):
                    js = slice((2 * p + h) * FT, (2 * p + h + 1) * FT)
                    hs = slice(h * FT, (h + 1) * FT)
                    dhi = ps_d.tile([MOUT - 128, FT], f32, name="dhi",
                                    tag="dhi")
                    nc.tensor.matmul(dhi[:], lhsT=uclo[:, 128:MOUT],
                                     rhs=mon_lo[:, js], start=True, stop=False)
                    nc.tensor.matmul(dhi[:], lhsT=uchi[:, 128:MOUT],
                                     rhs=mon_hi[0:KHI, js],
                                     start=False, stop=True)
                    dwh = work.tile([MOUT - 128, FT], f16, name="dwh",
                                    tag="dwh")
                    nc.vector.tensor_mul(dwh[:], dhi[:], wrep_hi[:, js])
                    nc.tensor.matmul(pt[:, hs], lhsT=glo, rhs=dwl[:, hs],
                                     start=True, stop=False)
                    nc.tensor.matmul(pt[:, hs], lhsT=ghi, rhs=dwh[:],
                                     start=False, stop=True)
                nc.scalar.activation(termSB[:, jp], pt[:], AF.Copy)

            if stage <= 5:
                nc.sync.dma_start(out=out_d, in_=outSB)
                return

            for o in range(MD):
                dst = termT[:, o * nloc:(o + 1) * nloc]
                src = termSB[o:o + 1, :]
                if o % 2 == 0:
                    nc.sync.dma_start(out=dst, in_=src)
                else:
                    nc.gpsimd.dma_start(out=dst, in_=src)

            for l, (c0, c1) in enumerate(lblk):
                w_l = wct[:, l * CD:(l + 1) * CD]
                for s0 in range(c0, c1, FT):
                    s1 = min(s0 + FT, c1)
                    pf = ps_f.tile([CD, FT], f32, name="pf", tag="f")
                    nc.tensor.matmul(pf[:, :s1 - s0], lhsT=w_l,
                                     rhs=termT[:, s0:s1], start=True, stop=True)
                    if l == 0:
                        nc.scalar.activation(outSB[:, s0:s1], pf[:, :s1 - s0],
                                             AF.Identity, bias=sb_b2)
                    else:
                        nc.scalar.activation(outSB[:, s0:s1], pf[:, :s1 - s0],
                                             AF.Copy)
            nc.sync.dma_start(out=out_d, in_=outSB)

          if repeat > 1:
              with tc.For_i(0, repeat, 1):
                  _emit()
          else:
              _emit()

    return nc


def _get_program(nloc, repeat=1, stage=6):
    key = (nloc, repeat, stage)
    if key not in _PROGRAM:
        nc = _build_program(nloc, repeat, stage)
        nc.compile()
        _PROGRAM[key] = nc
    return _PROGRAM[key]


def make_in_maps(irreps_x, atomic_numbers, w_fc1, b_fc1, U3, W3, U2, W2, U1, W1,
                 w_lin, w_fc2, b_fc2, nloc=NLOC, ncores=NCORES):
    irreps_x = np.asarray(irreps_x, np.float32)
    a_n = np.asarray(atomic_numbers).astype(np.int64)
    U3c, U2c = _sym_compress(np.asarray(U3, np.float64),
                             np.asarray(U2, np.float64))
    UA, UB, G = _build_ucomb_g(U3c, U2c, np.asarray(U1, np.float32))
    sel1 = _build_sel1()
    w_comb = np.einsum('lde,lec->ldc', np.asarray(w_fc2, np.float32),
                       np.asarray(w_lin, np.float32))
    w1t = np.concatenate([np.asarray(w_fc1, np.float32)[l].T for l in range(3)],
                         axis=1)
    wct = np.concatenate([w_comb[l].T for l in range(3)], axis=1)
    w3g = np.asarray(W3, np.float32)[a_n]
    w2g = np.asarray(W2, np.float32)[a_n]
    w1g = np.asarray(W1, np.float32)[a_n]
    F = nloc * CD

    def put(buf, nm, arr, r0=0):
        o = PK_OFF[nm]
        arr = np.asarray(arr, np.float32).astype(np.float16)
        buf[r0:r0 + arr.shape[0], o:o + arr.shape[1]] = arr

    b12 = np.stack([np.asarray(b_fc1, np.float32),
                    np.asarray(b_fc2, np.float32)], axis=1).astype(np.float32)
    in_maps = []
    for core in range(ncores):
        s = slice(core * nloc, (core + 1) * nloc)
        parts = []
        for l in range(3):
            seg = irreps_x[s, l * l:(l + 1) * (l + 1), :]
            parts.append(seg.transpose(2, 1, 0).reshape(CD, -1))
        xtc = np.concatenate(parts, axis=1)
        pk = np.zeros((128, PK_BASE + MD * nloc), np.float16)
        put(pk, "uclo", UA)
        put(pk, "uchi", UB)
        put(pk, "glo", G[0:128])
        put(pk, "ghi", G[128:MOUT])
        put(pk, "w1t", w1t)
        put(pk, "wct", wct)
        put(pk, "sel1", sel1)
        pk[:CD, PK_BASE:PK_BASE + MD * nloc] = xtc.astype(np.float16)
        wg3 = w3g[s].transpose(1, 2, 0).reshape(P3D, F)
        wg2 = w2g[s].transpose(1, 2, 0).reshape(P2D, F)
        wg1 = w1g[s].transpose(1, 2, 0).reshape(P1D, F)
        wrep = np.concatenate([np.tile(wg3, (MD, 1)), np.tile(wg2, (MD, 1)),
                               np.tile(wg1, (MD, 1))], axis=0)
        in_maps.append({
            "pk": pk,
            "b12": b12,
            "wrep": wrep.astype(np.float16),
        })
    return in_maps


def unpack_out(o, nloc=NLOC):
    return np.ascontiguousarray(
        o.reshape(CD, MD, nloc).transpose(2, 1, 0)).astype(np.float32)


def kernel(**inputs):
    from concourse import bass_utils
    in_maps = make_in_maps(**inputs)
    nc = _get_program(NLOC)
    res = bass_utils.run_bass_kernel_spmd(nc, in_maps,
                                          core_ids=list(range(NCORES)))
    outs = [unpack_out(res.results[c]["out"]) for c in range(NCORES)]
    return np.concatenate(outs, axis=0).astype(np.float32)


# revision 28
# speedup vs baseline: 1.7714x; 1.0365x over previous
import numpy as np

NTOT, MD, CD = 1024, 9, 64
P3D, P2D, P1D = 16, 4, 2
NCORES = 8
NLOC = NTOT // NCORES
FT = 512

PAIRS = [(a, a + d) for d in range(MD) for a in range(MD - d)]
TRIPS = [(a, a + d, a + d + e) for e in range(MD) for d in range(MD - e)
         for a in range(MD - d - e)]
N2, N3 = len(PAIRS), len(TRIPS)
QIDX = {q: k for k, q in enumerate(PAIRS)}
DOFF = [0]
for _d in range(MD):
    DOFF.append(DOFF[-1] + (MD - _d))
E0 = N2
E1 = N3 - E0
KMON = MD + N2 + N3
MOUT = MD * (P3D + P2D + P1D)
KLO = E1
KHI = 109

_PK_ITEMS = (("uclo", MOUT), ("uchi", MOUT), ("glo", MD), ("ghi", MD),
             ("w1t", 3 * CD), ("wct", 3 * CD), ("sel1", E1))
PK_OFF = {}
_c = 0
for _nm, _w in _PK_ITEMS:
    PK_OFF[_nm] = _c
    _c += _w
PK_BASE = _c
W9 = MD * NLOC

_PROGRAM = {}


def _sym_compress(U3, U2):
    tidx = {t: k for k, t in enumerate(TRIPS)}
    U3c = np.zeros((MD, N3, P3D), np.float64)
    for a in range(MD):
        for b in range(MD):
            for i in range(MD):
                U3c[:, tidx[tuple(sorted((a, b, i)))], :] += U3[:, a, b, i, :]
    U2c = np.zeros((MD, N2, P2D), np.float64)
    for a in range(MD):
        for i in range(MD):
            U2c[:, QIDX[tuple(sorted((a, i)))], :] += U2[:, a, i, :]
    return U3c.astype(np.float32), U2c.astype(np.float32)


def _build_ucomb_g(U3c, U2c, U1):
    U = np.zeros((KMON, MOUT), np.float32)
    G = np.zeros((MOUT, MD), np.float32)
    for o in range(MD):
        U[MD + N2:, o * P3D:(o + 1) * P3D] = U3c[o]
        U[MD:MD + N2, 144 + o * P2D:144 + (o + 1) * P2D] = U2c[o]
        U[0:MD, 180 + o * P1D:180 + (o + 1) * P1D] = U1[o]
        G[o * P3D:(o + 1) * P3D, o] = 1.0
        G[144 + o * P2D:144 + (o + 1) * P2D, o] = 1.0
        G[180 + o * P1D:180 + (o + 1) * P1D, o] = 1.0
    UA = U[MD + N2 + E0:]
    UB = np.zeros((KHI, MOUT), np.float32)
    UB[0:N2] = U[MD:MD + N2]
    UB[N2:N2 + MD] = U[0:MD]
    UB[64:64 + E0] = U[MD + N2:MD + N2 + E0]
    return UA, UB, G


def _build_sel1():
    sel1 = np.zeros((N2, E1), np.float32)
    for t, (a, b, i) in enumerate(TRIPS[E0:]):
        sel1[QIDX[(a, b)], t] = 1.0
    return sel1


def _build_program(nloc, repeat=1, stage=6):
    import concourse.bacc as bacc
    from concourse import mybir
    from concourse.tile import TileContext

    f16 = mybir.dt.float16
    f32 = mybir.dt.float32
    AF = mybir.ActivationFunctionType
    F = nloc * CD
    nft = F // FT
    w9 = MD * nloc
    lblk = [(0, nloc), (nloc, 4 * nloc), (4 * nloc, 9 * nloc)]
    pkw = PK_BASE + w9

    nc = bacc.Bacc("TRN2", debug=False, enable_asserts=False,
                   num_devices=NCORES, num_swdge_queues=4)

    pk_d = nc.dram_tensor("pk", [128, pkw], f16, kind="ExternalInput").ap()
    b12_d = nc.dram_tensor("b12", [CD, 2], f32, kind="ExternalInput").ap()
    wrep_d = nc.dram_tensor("wrep", [MOUT, F], f16, kind="ExternalInput").ap()
    out_d = nc.dram_tensor("out", [CD, w9], f32, kind="ExternalOutput").ap()
    x_d = nc.dram_tensor("x_sc", [MD, F], f16, kind="Internal").ap()
    xpre_d = nc.dram_tensor("xpre_sc", [N2, F], f16, kind="Internal").ap()
    xsuf_d = nc.dram_tensor("xsuf_sc", [N2, F], f16, kind="Internal").ap()

    FT2 = 2 * FT

    with TileContext(nc) as tc:
        with (
            tc.tile_pool(name="const", bufs=2) as const,
            tc.tile_pool(name="big", bufs=2) as big,
            tc.tile_pool(name="big1", bufs=1) as big1,
            tc.tile_pool(name="work", bufs=2) as work,
            tc.tile_pool(name="ps_r", bufs=1, space="PSUM") as ps_r,
            tc.tile_pool(name="ps_d", bufs=1, space="PSUM") as ps_d,
            tc.tile_pool(name="ps_t", bufs=1, space="PSUM") as ps_t,
        ):
          def _emit():
            pk = const.tile([128, pkw], f16, name="pk", tag="pk")
            pk32 = const.tile([CD, 2], f32, name="pk32", tag="pk32")

            def pks(nm, r0, r1):
                return pk[r0:r1, PK_OFF[nm]:PK_OFF[nm] + dict(_PK_ITEMS)[nm]]

            uclo = pks("uclo", 0, KLO)
            uchi = pks("uchi", 0, KHI)
            glo = pks("glo", 0, 128)
            ghi = pks("ghi", 0, MOUT - 128)
            w1t = pks("w1t", 0, CD)
            wct = pks("wct", 0, CD)
            sel1 = pks("sel1", 0, N2)
            xt = pk[0:CD, PK_BASE:PK_BASE + w9]
            sb_b1 = pk32[:, 0:1]
            sb_b2 = pk32[:, 1:2]

            ysb = big.tile([CD, w9], f16, name="ysb", tag="ysb")
            xsb = big.tile([MD, F], f16, name="xsb", tag="xsb")
            xpre = big.tile([N2, F], f16, name="xpre", tag="xpre")
            xsuf = big.tile([N2, F], f16, name="xsuf", tag="xsuf")
            xrep = big.tile([E1, F], f16, name="xrep", tag="xrep")
            mon_hi = big.tile([KHI, F], f16, name="mon_hi", tag="mon_hi")
            mon_lo = big1.tile([KLO, F], f16, name="mon_lo", tag="mon_lo")
            wrep_lo = big1.tile([128, F], f16, name="wrep_lo", tag="wrep_lo")
            wrep_hi = big1.tile([MOUT - 128, F], f16, name="wrep_hi",
                                tag="wrep_hi")
            termSB = big1.tile([MD, F], f16, name="termSB", tag="termSB")
            termT = big1.tile([CD, w9], f16, name="termT", tag="termT")
            outSB = big1.tile([CD, w9], f32, name="outSB", tag="outSB")

            nc.sync.dma_start(out=pk[:, :], in_=pk_d)
            nc.scalar.dma_start(out=pk32[:, :], in_=b12_d)
            if stage <= 5:
                nc.gpsimd.memset(outSB[:, :], 0.0)

            for l, (c0, c1) in enumerate(lblk):
                w_l = w1t[:, l * CD:(l + 1) * CD]
                for s0 in range(c0, c1, FT):
                    s1 = min(s0 + FT, c1)
                    py = ps_t.tile([CD, FT], f32, name="py", tag="t")
                    nc.tensor.matmul(py[:, :s1 - s0], lhsT=w_l,
                                     rhs=xt[:, s0:s1], start=True, stop=True)
                    if l == 0:
                        nc.scalar.activation(ysb[:, s0:s1], py[:, :s1 - s0],
                                             AF.Identity, bias=sb_b1)
                    else:
                        nc.scalar.activation(ysb[:, s0:s1], py[:, :s1 - s0],
                                             AF.Copy)

            if stage <= 1:
                nc.sync.dma_start(out=out_d, in_=outSB)
                return

            for m in range(MD):
                eng = (nc.sync, nc.scalar, nc.gpsimd)[m % 3]
                eng.dma_start(out=x_d[m:m + 1, :],
                              in_=ysb[:, m * nloc:(m + 1) * nloc])
            nc.gpsimd.dma_start(out=xsb[:, :], in_=x_d)
            nc.scalar.dma_start(out=mon_hi[N2:N2 + MD, :], in_=x_d)
            nc.gpsimd.dma_start(out=mon_hi[54:63, :], in_=x_d)
            nc.gpsimd.dma_start(out=mon_hi[63:64, :], in_=x_d[0:1, :])

            if stage <= 2:
                nc.sync.dma_start(out=out_d, in_=outSB)
                return

            for d in range(MD):
                q0, cnt = DOFF[d], MD - d
                nc.sync.dma_start(out=xpre_d[q0:q0 + cnt, :],
                                  in_=x_d[0:cnt, :])
                nc.scalar.dma_start(out=xsuf_d[q0:q0 + cnt, :],
                                    in_=x_d[d:MD, :])
            h2 = F // 2
            for c0, c1 in ((0, h2), (h2, F)):
                cs = slice(c0, c1)
                nc.sync.dma_start(out=xpre[:, cs], in_=xpre_d[:, c0:c1])
                nc.scalar.dma_start(out=xsuf[:, cs], in_=xsuf_d[:, c0:c1])
                t0 = 0
                for e in range(1, MD):
                    ln = N2 - DOFF[e]
                    eng = (nc.sync, nc.scalar)[e % 2]
                    eng.dma_start(out=xrep[t0:t0 + ln, cs],
                                  in_=xsuf_d[DOFF[e]:N2, c0:c1])
                    t0 += ln
                nc.vector.tensor_mul(mon_hi[0:N2, cs], xpre[:, cs],
                                     xsuf[:, cs])
                nc.vector.tensor_mul(mon_hi[64:64 + E0, cs],
                                     mon_hi[0:N2, cs], xsuf[:, cs])

            if stage <= 3:
                nc.sync.dma_start(out=out_d, in_=outSB)
                return

            if stage >= 5:
                h = F // 2
                nc.sync.dma_start(out=wrep_lo[:, 0:h], in_=wrep_d[0:128, 0:h])
                nc.scalar.dma_start(out=wrep_lo[:, h:F],
                                    in_=wrep_d[0:128, h:F])
                nc.sync.dma_start(out=wrep_hi[:, 0:h],
                                  in_=wrep_d[128:MOUT, 0:h])
                nc.scalar.dma_start(out=wrep_hi[:, h:F],
                                    in_=wrep_d[128:MOUT, h:F])

            for p in range(nft // 2):
                jp = slice(p * FT2, (p + 1) * FT2)
                ps1 = ps_r.tile([E1, FT2], f32, name="ps1", tag="r1")
                for h in range(2):
                    js = slice((2 * p + h) * FT, (2 * p + h + 1) * FT)
                    hs = slice(h * FT, (h + 1) * FT)
                    nc.tensor.matmul(ps1[:, hs], lhsT=sel1,
                                     rhs=mon_hi[0:N2, js],
                                     start=True, stop=True)
                nc.vector.tensor_mul(mon_lo[:, jp], xrep[:, jp], ps1[:])
                if stage <= 4:
                    continue
                dlo = ps_d.tile([128, FT2], f32, name="dlo", tag="dlo")
                pt = ps_t.tile([MD, FT2], f32, name="pt", tag="t")
                for h in range(2):
                    js = slice((2 * p + h) * FT, (2 * p + h + 1) * FT)
                    hs = slice(h * FT, (h + 1) * FT)
                    nc.tensor.matmul(dlo[:, hs], lhsT=uclo[:, 0:128],
                                     rhs=mon_lo[:, js], start=True, stop=False)
                    nc.tensor.matmul(dlo[:, hs], lhsT=uchi[:, 0:128],
                                     rhs=mon_hi[0:KHI, js],
                                     start=False, stop=True)
                dhi = ps_d.tile([MOUT - 128, FT2], f32, name="dhi",
                                tag="dhi")
                for h in range(2):
                    js = slice((2 * p + h) * FT, (2 * p + h + 1) * FT)
                    hs = slice(h * FT, (h + 1) * FT)
                    nc.tensor.matmul(dhi[:, hs], lhsT=uclo[:, 128:MOUT],
                                     rhs=mon_lo[:, js], start=True, stop=False)
                    nc.tensor.matmul(dhi[:, hs], lhsT=uchi[:, 128:MOUT],
                                     rhs=mon_hi[0:KHI, js],
                                     start=False, stop=True)
                dwl = work.tile([128, FT2], f16, name="dwl", tag="dwl")
                dwh = work.tile([MOUT - 128, FT2], f16, name="dwh", tag="dwh")
                nc.vector.tensor_mul(dwl[:], dlo[:], wrep_lo[:, jp])
                nc.vector.tensor_mul(dwh[:], dhi[:], wrep_hi[:, jp])
                for h in range(2):
                    hs = slice(h * FT, (h + 1) * FT)
                    nc.tensor.matmul(pt[:, hs], lhsT=glo, rhs=dwl[:, hs],
                                     start=True, stop=False)
                    nc.tensor.matmul(pt[:, hs], lhsT=ghi, rhs=dwh[:, hs],
                                     start=False, stop=True)
                nc.scalar.activation(termSB[:, jp], pt[:], AF.Copy)

            if stage <= 5:
                nc.sync.dma_start(out=out_d, in_=outSB)
                return

            for o in range(MD):
                dst = termT[:, o * nloc:(o + 1) * nloc]
                src = termSB[o:o + 1, :]
                if o % 2 == 0:
                    nc.sync.dma_start(out=dst, in_=src)
                else:
                    nc.gpsimd.dma_start(out=dst, in_=src)

            for l, (c0, c1) in enumerate(lblk):
                w_l = wct[:, l * CD:(l + 1) * CD]
                for s0 in range(c0, c1, FT):
                    s1 = min(s0 + FT, c1)
                    pf = ps_t.tile([CD, FT], f32, name="pf", tag="t")
                    nc.tensor.matmul(pf[:, :s1 - s0], lhsT=w_l,
                                     rhs=termT[:, s0:s1], start=True, stop=True)
                    if l == 0:
                        nc.scalar.activation(outSB[:, s0:s1], pf[:, :s1 - s0],
                                             AF.Identity, bias=sb_b2)
                    else:
                        nc.scalar.activation(outSB[:, s0:s1], pf[:, :s1 - s0],
                                             AF.Copy)
            nc.sync.dma_start(out=out_d, in_=outSB)

          if repeat > 1:
              with tc.For_i(0, repeat, 1):
                  _emit()
          else:
              _emit()

    return nc


def _get_program(nloc, repeat=1, stage=6):
    key = (nloc, repeat, stage)
    if key not in _PROGRAM:
        nc = _build_program(nloc, repeat, stage)
        nc.compile()
        _PROGRAM[key] = nc
    return _PROGRAM[key]


def make_in_maps(irreps_x, atomic_numbers, w_fc1, b_fc1, U3, W3, U2, W2, U1, W1,
                 w_lin, w_fc2, b_fc2, nloc=NLOC, ncores=NCORES):
    irreps_x = np.asarray(irreps_x, np.float32)
    a_n = np.asarray(atomic_numbers).astype(np.int64)
    U3c, U2c = _sym_compress(np.asarray(U3, np.float64),
                             np.asarray(U2, np.float64))
    UA, UB, G = _build_ucomb_g(U3c, U2c, np.asarray(U1, np.float32))
    sel1 = _build_sel1()
    w_comb = np.einsum('lde,lec->ldc', np.asarray(w_fc2, np.float32),
                       np.asarray(w_lin, np.float32))
    w1t = np.concatenate([np.asarray(w_fc1, np.float32)[l].T for l in range(3)],
                         axis=1)
    wct = np.concatenate([w_comb[l].T for l in range(3)], axis=1)
    w3g = np.asarray(W3, np.float32)[a_n]
    w2g = np.asarray(W2, np.float32)[a_n]
    w1g = np.asarray(W1, np.float32)[a_n]
    F = nloc * CD

    def put(buf, nm, arr, r0=0):
        o = PK_OFF[nm]
        arr = np.asarray(arr, np.float32).astype(np.float16)
        buf[r0:r0 + arr.shape[0], o:o + arr.shape[1]] = arr

    b12 = np.stack([np.asarray(b_fc1, np.float32),
                    np.asarray(b_fc2, np.float32)], axis=1).astype(np.float32)
    in_maps = []
    for core in range(ncores):
        s = slice(core * nloc, (core + 1) * nloc)
        parts = []
        for l in range(3):
            seg = irreps_x[s, l * l:(l + 1) * (l + 1), :]
            parts.append(seg.transpose(2, 1, 0).reshape(CD, -1))
        xtc = np.concatenate(parts, axis=1)
        pk = np.zeros((128, PK_BASE + MD * nloc), np.float16)
        put(pk, "uclo", UA)
        put(pk, "uchi", UB)
        put(pk, "glo", G[0:128])
        put(pk, "ghi", G[128:MOUT])
        put(pk, "w1t", w1t)
        put(pk, "wct", wct)
        put(pk, "sel1", sel1)
        pk[:CD, PK_BASE:PK_BASE + MD * nloc] = xtc.astype(np.float16)
        wg3 = w3g[s].transpose(1, 2, 0).reshape(P3D, F)
        wg2 = w2g[s].transpose(1, 2, 0).reshape(P2D, F)
        wg1 = w1g[s].transpose(1, 2, 0).reshape(P1D, F)
        wrep = np.concatenate([np.tile(wg3, (MD, 1)), np.tile(wg2, (MD, 1)),
                               np.tile(wg1, (MD, 1))], axis=0)
        in_maps.append({
            "pk": pk,
            "b12": b12,
            "wrep": wrep.astype(np.float16),
        })
    return in_maps


def unpack_out(o, nloc=NLOC):
    return np.ascontiguousarray(
        o.reshape(CD, MD, nloc).transpose(2, 1, 0)).astype(np.float32)


def kernel(**inputs):
    from concourse import bass_utils
    in_maps = make_in_maps(**inputs)
    nc = _get_program(NLOC)
    res = bass_utils.run_bass_kernel_spmd(nc, in_maps,
                                          core_ids=list(range(NCORES)))
    outs = [unpack_out(res.results[c]["out"]) for c in range(NCORES)]
    return np.concatenate(outs, axis=0).astype(np.float32)


# revision 30
# speedup vs baseline: 2.1990x; 1.2414x over previous
import numpy as np

NTOT, MD, CD = 1024, 9, 64
P3D, P2D, P1D = 16, 4, 2
NCORES = 8
NLOC = NTOT // NCORES
FT = 512

PAIRS = [(a, a + d) for d in range(MD) for a in range(MD - d)]
TRIPS = [(a, a + d, a + d + e) for e in range(MD) for d in range(MD - e)
         for a in range(MD - d - e)]
N2, N3 = len(PAIRS), len(TRIPS)
QIDX = {q: k for k, q in enumerate(PAIRS)}
DOFF = [0]
for _d in range(MD):
    DOFF.append(DOFF[-1] + (MD - _d))
E0 = N2
E1 = N3 - E0
KMON = MD + N2 + N3
MOUT = MD * (P3D + P2D + P1D)
KLO = E1
KHI = 109

_PK_ITEMS = (("uclo", MOUT), ("uchi", MOUT), ("glo", MD), ("ghi", MD),
             ("w1t", 3 * CD), ("wct", 3 * CD), ("sel1", E1))
PK_OFF = {}
_c = 0
for _nm, _w in _PK_ITEMS:
    PK_OFF[_nm] = _c
    _c += _w
PK_BASE = _c
W9 = MD * NLOC

_PROGRAM = {}


def _sym_compress(U3, U2):
    tidx = {t: k for k, t in enumerate(TRIPS)}
    U3c = np.zeros((MD, N3, P3D), np.float64)
    for a in range(MD):
        for b in range(MD):
            for i in range(MD):
                U3c[:, tidx[tuple(sorted((a, b, i)))], :] += U3[:, a, b, i, :]
    U2c = np.zeros((MD, N2, P2D), np.float64)
    for a in range(MD):
        for i in range(MD):
            U2c[:, QIDX[tuple(sorted((a, i)))], :] += U2[:, a, i, :]
    return U3c.astype(np.float32), U2c.astype(np.float32)


def _build_ucomb_g(U3c, U2c, U1):
    U = np.zeros((KMON, MOUT), np.float32)
    G = np.zeros((MOUT, MD), np.float32)
    for o in range(MD):
        U[MD + N2:, o * P3D:(o + 1) * P3D] = U3c[o]
        U[MD:MD + N2, 144 + o * P2D:144 + (o + 1) * P2D] = U2c[o]
        U[0:MD, 180 + o * P1D:180 + (o + 1) * P1D] = U1[o]
        G[o * P3D:(o + 1) * P3D, o] = 1.0
        G[144 + o * P2D:144 + (o + 1) * P2D, o] = 1.0
        G[180 + o * P1D:180 + (o + 1) * P1D, o] = 1.0
    UA = U[MD + N2 + E0:]
    UB = np.zeros((KHI, MOUT), np.float32)
    UB[0:N2] = U[MD:MD + N2]
    UB[N2:N2 + MD] = U[0:MD]
    UB[64:64 + E0] = U[MD + N2:MD + N2 + E0]
    return UA, UB, G


def _build_sel1():
    sel1 = np.zeros((N2, E1), np.float32)
    for t, (a, b, i) in enumerate(TRIPS[E0:]):
        sel1[QIDX[(a, b)], t] = 1.0
    return sel1


def _build_program(nloc, repeat=1, stage=6):
    import concourse.bacc as bacc
    from concourse import mybir
    from concourse.tile import TileContext

    f16 = mybir.dt.float16
    f32 = mybir.dt.float32
    AF = mybir.ActivationFunctionType
    F = nloc * CD
    nft = F // FT
    w9 = MD * nloc
    lblk = [(0, nloc), (nloc, 4 * nloc), (4 * nloc, 9 * nloc)]
    pkw = PK_BASE + w9

    nc = bacc.Bacc("TRN2", debug=False, enable_asserts=False,
                   num_devices=NCORES, num_swdge_queues=4)

    pk_d = nc.dram_tensor("pk", [128, pkw], f16, kind="ExternalInput").ap()
    b12_d = nc.dram_tensor("b12", [CD, 2], f32, kind="ExternalInput").ap()
    wrep_d = nc.dram_tensor("wrep", [MOUT, F], f16, kind="ExternalInput").ap()
    out_d = nc.dram_tensor("out", [CD, w9], f32, kind="ExternalOutput").ap()
    x_d = nc.dram_tensor("x_sc", [MD, F], f16, kind="Internal").ap()
    xpre_d = nc.dram_tensor("xpre_sc", [N2, F], f16, kind="Internal").ap()
    xsuf_d = nc.dram_tensor("xsuf_sc", [N2, F], f16, kind="Internal").ap()

    FT2 = 2 * FT

    with TileContext(nc) as tc:
        with (
            tc.tile_pool(name="const", bufs=1) as const,
            tc.tile_pool(name="big", bufs=1) as big,
            tc.tile_pool(name="big1", bufs=1) as big1,
            tc.tile_pool(name="work", bufs=2) as work,
            tc.tile_pool(name="ps_r", bufs=1, space="PSUM") as ps_r,
            tc.tile_pool(name="ps_d", bufs=1, space="PSUM") as ps_d,
            tc.tile_pool(name="ps_t", bufs=1, space="PSUM") as ps_t,
        ):
          def _emit():
            pk = const.tile([128, pkw], f16, name="pk", tag="pk")
            pk32 = const.tile([CD, 2], f32, name="pk32", tag="pk32")

            def pks(nm, r0, r1):
                return pk[r0:r1, PK_OFF[nm]:PK_OFF[nm] + dict(_PK_ITEMS)[nm]]

            uclo = pks("uclo", 0, KLO)
            uchi = pks("uchi", 0, KHI)
            glo = pks("glo", 0, 128)
            ghi = pks("ghi", 0, MOUT - 128)
            w1t = pks("w1t", 0, CD)
            wct = pks("wct", 0, CD)
            sel1 = pks("sel1", 0, N2)
            xt = pk[0:CD, PK_BASE:PK_BASE + w9]
            sb_b1 = pk32[:, 0:1]
            sb_b2 = pk32[:, 1:2]

            ysb = big.tile([CD, w9], f16, name="ysb", tag="ysb")
            xsb = big.tile([MD, F], f16, name="xsb", tag="xsb")
            xpre = big.tile([N2, F], f16, name="xpre", tag="xpre")
            xsuf = big.tile([N2, F], f16, name="xsuf", tag="xsuf")
            xrep = big.tile([E1, F], f16, name="xrep", tag="xrep")
            mon_hi = big.tile([KHI, F], f16, name="mon_hi", tag="mon_hi")
            mon_lo = big1.tile([KLO, F], f16, name="mon_lo", tag="mon_lo")
            wrep_lo = big1.tile([128, F], f16, name="wrep_lo", tag="wrep_lo")
            wrep_hi = big1.tile([MOUT - 128, F], f16, name="wrep_hi",
                                tag="wrep_hi")
            termSB = big1.tile([MD, F], f16, name="termSB", tag="termSB")
            termT = big1.tile([CD, w9], f16, name="termT", tag="termT")
            outSB = big1.tile([CD, w9], f32, name="outSB", tag="outSB")

            nc.sync.dma_start(out=pk[:, :], in_=pk_d)
            nc.scalar.dma_start(out=pk32[:, :], in_=b12_d)
            if stage <= 5:
                nc.gpsimd.memset(outSB[:, :], 0.0)

            for l, (c0, c1) in enumerate(lblk):
                w_l = w1t[:, l * CD:(l + 1) * CD]
                for s0 in range(c0, c1, FT):
                    s1 = min(s0 + FT, c1)
                    py = ps_t.tile([CD, FT], f32, name="py", tag="t")
                    nc.tensor.matmul(py[:, :s1 - s0], lhsT=w_l,
                                     rhs=xt[:, s0:s1], start=True, stop=True)
                    if l == 0:
                        nc.scalar.activation(ysb[:, s0:s1], py[:, :s1 - s0],
                                             AF.Identity, bias=sb_b1)
                    else:
                        nc.scalar.activation(ysb[:, s0:s1], py[:, :s1 - s0],
                                             AF.Copy)

            if stage <= 1:
                nc.sync.dma_start(out=out_d, in_=outSB)
                return

            for m in range(MD):
                eng = (nc.sync, nc.scalar, nc.gpsimd)[m % 3]
                eng.dma_start(out=x_d[m:m + 1, :],
                              in_=ysb[:, m * nloc:(m + 1) * nloc])
            nc.gpsimd.dma_start(out=xsb[:, :], in_=x_d)
            nc.scalar.dma_start(out=mon_hi[N2:N2 + MD, :], in_=x_d)
            nc.gpsimd.dma_start(out=mon_hi[54:63, :], in_=x_d)
            nc.gpsimd.dma_start(out=mon_hi[63:64, :], in_=x_d[0:1, :])

            if stage <= 2:
                nc.sync.dma_start(out=out_d, in_=outSB)
                return

            for d in range(MD):
                q0, cnt = DOFF[d], MD - d
                nc.sync.dma_start(out=xpre_d[q0:q0 + cnt, :],
                                  in_=x_d[0:cnt, :])
                nc.gpsimd.dma_start(out=xsuf_d[q0:q0 + cnt, :],
                                     in_=x_d[d:MD, :])
            h2 = F // 2
            for c0, c1 in ((0, h2), (h2, F)):
                cs = slice(c0, c1)
                nc.sync.dma_start(out=xpre[:, cs], in_=xpre_d[:, c0:c1])
                nc.scalar.dma_start(out=xsuf[:, cs], in_=xsuf_d[:, c0:c1])
                t0 = 0
                for e in range(1, MD):
                    ln = N2 - DOFF[e]
                    eng = (nc.sync, nc.scalar)[e % 2]
                    eng.dma_start(out=xrep[t0:t0 + ln, cs],
                                  in_=xsuf_d[DOFF[e]:N2, c0:c1])
                    t0 += ln
                nc.vector.tensor_mul(mon_hi[0:N2, cs], xpre[:, cs],
                                     xsuf[:, cs])
                nc.vector.tensor_mul(mon_hi[64:64 + E0, cs],
                                     mon_hi[0:N2, cs], xsuf[:, cs])

            if stage <= 3:
                nc.sync.dma_start(out=out_d, in_=outSB)
                return

            if stage >= 5:
                h = F // 2
                nc.gpsimd.dma_start(out=wrep_lo[:, 0:h],
                                    in_=wrep_d[0:128, 0:h])
                nc.gpsimd.dma_start(out=wrep_lo[:, h:F],
                                    in_=wrep_d[0:128, h:F])
                nc.gpsimd.dma_start(out=wrep_hi[:, 0:h],
                                    in_=wrep_d[128:MOUT, 0:h])
                nc.gpsimd.dma_start(out=wrep_hi[:, h:F],
                                    in_=wrep_d[128:MOUT, h:F])

            for p in range(nft // 2):
                jp = slice(p * FT2, (p + 1) * FT2)
                ps1 = ps_r.tile([E1, FT2], f32, name="ps1", tag="r1")
                for h in range(2):
                    js = slice((2 * p + h) * FT, (2 * p + h + 1) * FT)
                    hs = slice(h * FT, (h + 1) * FT)
                    nc.tensor.matmul(ps1[:, hs], lhsT=sel1,
                                     rhs=mon_hi[0:N2, js],
                                     start=True, stop=True)
                nc.vector.tensor_mul(mon_lo[:, jp], xrep[:, jp], ps1[:])
                if stage <= 4:
                    continue
                dlo = ps_d.tile([128, FT2], f32, name="dlo", tag="dlo")
                pt = ps_t.tile([MD, FT2], f32, name="pt", tag="t")
                for h in range(2):
                    js = slice((2 * p + h) * FT, (2 * p + h + 1) * FT)
                    hs = slice(h * FT, (h + 1) * FT)
                    nc.tensor.matmul(dlo[:, hs], lhsT=uclo[:, 0:128],
                                     rhs=mon_lo[:, js], start=True, stop=False)
                    nc.tensor.matmul(dlo[:, hs], lhsT=uchi[:, 0:128],
                                     rhs=mon_hi[0:KHI, js],
                                     start=False, stop=True)
                dhi = ps_d.tile([MOUT - 128, FT2], f32, name="dhi",
                                tag="dhi")
                for h in range(2):
                    js = slice((2 * p + h) * FT, (2 * p + h + 1) * FT)
                    hs = slice(h * FT, (h + 1) * FT)
                    nc.tensor.matmul(dhi[:, hs], lhsT=uclo[:, 128:MOUT],
                                     rhs=mon_lo[:, js], start=True, stop=False)
                    nc.tensor.matmul(dhi[:, hs], lhsT=uchi[:, 128:MOUT],
                                     rhs=mon_hi[0:KHI, js],
                                     start=False, stop=True)
                dsl = work.tile([128, FT2], f16, name="dsl", tag="dsl")
                dsh = work.tile([MOUT - 128, FT2], f16, name="dsh", tag="dsh")
                nc.scalar.activation(dsl[:], dlo[:], AF.Copy)
                nc.scalar.activation(dsh[:], dhi[:], AF.Copy)
                dwl = work.tile([128, FT2], f16, name="dwl", tag="dwl")
                dwh = work.tile([MOUT - 128, FT2], f16, name="dwh", tag="dwh")
                nc.vector.tensor_mul(dwl[:], dsl[:], wrep_lo[:, jp])
                nc.vector.tensor_mul(dwh[:], dsh[:], wrep_hi[:, jp])
                for h in range(2):
                    hs = slice(h * FT, (h + 1) * FT)
                    nc.tensor.matmul(pt[:, hs], lhsT=glo, rhs=dwl[:, hs],
                                     start=True, stop=False)
                    nc.tensor.matmul(pt[:, hs], lhsT=ghi, rhs=dwh[:, hs],
                                     start=False, stop=True)
                nc.scalar.activation(termSB[:, jp], pt[:], AF.Copy)

            if stage <= 5:
                nc.sync.dma_start(out=out_d, in_=outSB)
                return

            for o in range(MD):
                dst = termT[:, o * nloc:(o + 1) * nloc]
                src = termSB[o:o + 1, :]
                if o % 2 == 0:
                    nc.sync.dma_start(out=dst, in_=src)
                else:
                    nc.gpsimd.dma_start(out=dst, in_=src)

            for l, (c0, c1) in enumerate(lblk):
                w_l = wct[:, l * CD:(l + 1) * CD]
                for s0 in range(c0, c1, FT):
                    s1 = min(s0 + FT, c1)
                    pf = ps_t.tile([CD, FT], f32, name="pf", tag="t")
                    nc.tensor.matmul(pf[:, :s1 - s0], lhsT=w_l,
                                     rhs=termT[:, s0:s1], start=True, stop=True)
                    if l == 0:
                        nc.scalar.activation(outSB[:, s0:s1], pf[:, :s1 - s0],
                                             AF.Identity, bias=sb_b2)
                    else:
                        nc.scalar.activation(outSB[:, s0:s1], pf[:, :s1 - s0],
                                             AF.Copy)
            nc.sync.dma_start(out=out_d, in_=outSB)

          if repeat > 1:
              with tc.For_i(0, repeat, 1):
                  _emit()
          else:
              _emit()

    return nc


def _get_program(nloc, repeat=1, stage=6):
    key = (nloc, repeat, stage)
    if key not in _PROGRAM:
        nc = _build_program(nloc, repeat, stage)
        nc.compile()
        _PROGRAM[key] = nc
    return _PROGRAM[key]


def make_in_maps(irreps_x, atomic_numbers, w_fc1, b_fc1, U3, W3, U2, W2, U1, W1,
                 w_lin, w_fc2, b_fc2, nloc=NLOC, ncores=NCORES):
    irreps_x = np.asarray(irreps_x, np.float32)
    a_n = np.asarray(atomic_numbers).astype(np.int64)
    U3c, U2c = _sym_compress(np.asarray(U3, np.float64),
                             np.asarray(U2, np.float64))
    UA, UB, G = _build_ucomb_g(U3c, U2c, np.asarray(U1, np.float32))
    sel1 = _build_sel1()
    w_comb = np.einsum('lde,lec->ldc', np.asarray(w_fc2, np.float32),
                       np.asarray(w_lin, np.float32))
    w1t = np.concatenate([np.asarray(w_fc1, np.float32)[l].T for l in range(3)],
                         axis=1)
    wct = np.concatenate([w_comb[l].T for l in range(3)], axis=1)
    w3g = np.asarray(W3, np.float32)[a_n]
    w2g = np.asarray(W2, np.float32)[a_n]
    w1g = np.asarray(W1, np.float32)[a_n]
    F = nloc * CD

    def put(buf, nm, arr, r0=0):
        o = PK_OFF[nm]
        arr = np.asarray(arr, np.float32).astype(np.float16)
        buf[r0:r0 + arr.shape[0], o:o + arr.shape[1]] = arr

    b12 = np.stack([np.asarray(b_fc1, np.float32),
                    np.asarray(b_fc2, np.float32)], axis=1).astype(np.float32)
    in_maps = []
    for core in range(ncores):
        s = slice(core * nloc, (core + 1) * nloc)
        parts = []
        for l in range(3):
            seg = irreps_x[s, l * l:(l + 1) * (l + 1), :]
            parts.append(seg.transpose(2, 1, 0).reshape(CD, -1))
        xtc = np.concatenate(parts, axis=1)
        pk = np.zeros((128, PK_BASE + MD * nloc), np.float16)
        put(pk, "uclo", UA)
        put(pk, "uchi", UB)
        put(pk, "glo", G[0:128])
        put(pk, "ghi", G[128:MOUT])
        put(pk, "w1t", w1t)
        put(pk, "wct", wct)
        put(pk, "sel1", sel1)
        pk[:CD, PK_BASE:PK_BASE + MD * nloc] = xtc.astype(np.float16)
        wg3 = w3g[s].transpose(1, 2, 0).reshape(P3D, F)
        wg2 = w2g[s].transpose(1, 2, 0).reshape(P2D, F)
        wg1 = w1g[s].transpose(1, 2, 0).reshape(P1D, F)
        wrep = np.concatenate([np.tile(wg3, (MD, 1)), np.tile(wg2, (MD, 1)),
                               np.tile(wg1, (MD, 1))], axis=0)
        in_maps.append({
            "pk": pk,
            "b12": b12,
            "wrep": wrep.astype(np.float16),
        })
    return in_maps


def unpack_out(o, nloc=NLOC):
    return np.ascontiguousarray(
        o.reshape(CD, MD, nloc).transpose(2, 1, 0)).astype(np.float32)


def kernel(**inputs):
    from concourse import bass_utils
    in_maps = make_in_maps(**inputs)
    nc = _get_program(NLOC)
    res = bass_utils.run_bass_kernel_spmd(nc, in_maps,
                                          core_ids=list(range(NCORES)))
    outs = [unpack_out(res.results[c]["out"]) for c in range(NCORES)]
    return np.concatenate(outs, axis=0).astype(np.float32)


# revision 31
# speedup vs baseline: 2.2517x; 1.0240x over previous
import numpy as np

NTOT, MD, CD = 1024, 9, 64
P3D, P2D, P1D = 16, 4, 2
NCORES = 8
NLOC = NTOT // NCORES
FT = 512

PAIRS = [(a, a + d) for d in range(MD) for a in range(MD - d)]
TRIPS = [(a, a + d, a + d + e) for e in range(MD) for d in range(MD - e)
         for a in range(MD - d - e)]
N2, N3 = len(PAIRS), len(TRIPS)
QIDX = {q: k for k, q in enumerate(PAIRS)}
DOFF = [0]
for _d in range(MD):
    DOFF.append(DOFF[-1] + (MD - _d))
E0 = N2
E1 = N3 - E0
KMON = MD + N2 + N3
MOUT = MD * (P3D + P2D + P1D)
KLO = E1
KHI = 109

_PK_ITEMS = (("uclo", MOUT), ("uchi", MOUT), ("glo", MD), ("ghi", MD),
             ("w1t", 3 * CD), ("wct", 3 * CD), ("sel1", E1))
PK_OFF = {}
_c = 0
for _nm, _w in _PK_ITEMS:
    PK_OFF[_nm] = _c
    _c += _w
PK_BASE = _c
W9 = MD * NLOC

_PROGRAM = {}


def _sym_compress(U3, U2):
    tidx = {t: k for k, t in enumerate(TRIPS)}
    U3c = np.zeros((MD, N3, P3D), np.float64)
    for a in range(MD):
        for b in range(MD):
            for i in range(MD):
                U3c[:, tidx[tuple(sorted((a, b, i)))], :] += U3[:, a, b, i, :]
    U2c = np.zeros((MD, N2, P2D), np.float64)
    for a in range(MD):
        for i in range(MD):
            U2c[:, QIDX[tuple(sorted((a, i)))], :] += U2[:, a, i, :]
    return U3c.astype(np.float32), U2c.astype(np.float32)


def _build_ucomb_g(U3c, U2c, U1):
    U = np.zeros((KMON, MOUT), np.float32)
    G = np.zeros((MOUT, MD), np.float32)
    for o in range(MD):
        U[MD + N2:, o * P3D:(o + 1) * P3D] = U3c[o]
        U[MD:MD + N2, 144 + o * P2D:144 + (o + 1) * P2D] = U2c[o]
        U[0:MD, 180 + o * P1D:180 + (o + 1) * P1D] = U1[o]
        G[o * P3D:(o + 1) * P3D, o] = 1.0
        G[144 + o * P2D:144 + (o + 1) * P2D, o] = 1.0
        G[180 + o * P1D:180 + (o + 1) * P1D, o] = 1.0
    UA = U[MD + N2 + E0:]
    UB = np.zeros((KHI, MOUT), np.float32)
    UB[0:N2] = U[MD:MD + N2]
    UB[N2:N2 + MD] = U[0:MD]
    UB[64:64 + E0] = U[MD + N2:MD + N2 + E0]
    return UA, UB, G


def _build_sel1():
    sel1 = np.zeros((N2, E1), np.float32)
    for t, (a, b, i) in enumerate(TRIPS[E0:]):
        sel1[QIDX[(a, b)], t] = 1.0
    return sel1


def _build_program(nloc, repeat=1, stage=6):
    import concourse.bacc as bacc
    from concourse import mybir
    from concourse.tile import TileContext

    f16 = mybir.dt.float16
    f32 = mybir.dt.float32
    AF = mybir.ActivationFunctionType
    F = nloc * CD
    nft = F // FT
    w9 = MD * nloc
    lblk = [(0, nloc), (nloc, 4 * nloc), (4 * nloc, 9 * nloc)]
    pkw = PK_BASE + w9

    nc = bacc.Bacc("TRN2", debug=False, enable_asserts=False,
                   num_devices=NCORES, num_swdge_queues=4)

    pk_d = nc.dram_tensor("pk", [128, pkw], f16, kind="ExternalInput").ap()
    b12_d = nc.dram_tensor("b12", [CD, 2], f32, kind="ExternalInput").ap()
    wrep_d = nc.dram_tensor("wrep", [MOUT, F], f16, kind="ExternalInput").ap()
    out_d = nc.dram_tensor("out", [CD, w9], f32, kind="ExternalOutput").ap()
    x_d = nc.dram_tensor("x_sc", [MD, F], f16, kind="Internal").ap()
    xpre_d = nc.dram_tensor("xpre_sc", [N2, F], f16, kind="Internal").ap()
    xsuf_d = nc.dram_tensor("xsuf_sc", [N2, F], f16, kind="Internal").ap()

    FT2 = 2 * FT

    with TileContext(nc) as tc:
        with (
            tc.tile_pool(name="const", bufs=1) as const,
            tc.tile_pool(name="big", bufs=1) as big,
            tc.tile_pool(name="big1", bufs=1) as big1,
            tc.tile_pool(name="work", bufs=2) as work,
            tc.tile_pool(name="ps_r", bufs=1, space="PSUM") as ps_r,
            tc.tile_pool(name="ps_d", bufs=1, space="PSUM") as ps_d,
            tc.tile_pool(name="ps_t", bufs=1, space="PSUM") as ps_t,
        ):
          def _emit():
            pk = const.tile([128, pkw], f16, name="pk", tag="pk")
            pk32 = const.tile([CD, 2], f32, name="pk32", tag="pk32")

            def pks(nm, r0, r1):
                return pk[r0:r1, PK_OFF[nm]:PK_OFF[nm] + dict(_PK_ITEMS)[nm]]

            uclo = pks("uclo", 0, KLO)
            uchi = pks("uchi", 0, KHI)
            glo = pks("glo", 0, 128)
            ghi = pks("ghi", 0, MOUT - 128)
            w1t = pks("w1t", 0, CD)
            wct = pks("wct", 0, CD)
            sel1 = pks("sel1", 0, N2)
            xt = pk[0:CD, PK_BASE:PK_BASE + w9]
            sb_b1 = pk32[:, 0:1]
            sb_b2 = pk32[:, 1:2]

            ysb = big.tile([CD, w9], f16, name="ysb", tag="ysb")
            xsb = big.tile([MD, F], f16, name="xsb", tag="xsb")
            xpre = big.tile([N2, F], f16, name="xpre", tag="xpre")
            xsuf = big.tile([N2, F], f16, name="xsuf", tag="xsuf")
            xrep = big.tile([E1, F], f16, name="xrep", tag="xrep")
            mon_hi = big.tile([KHI, F], f16, name="mon_hi", tag="mon_hi")
            mon_lo = big1.tile([KLO, F], f16, name="mon_lo", tag="mon_lo")
            wrep_lo = big1.tile([128, F], f16, name="wrep_lo", tag="wrep_lo")
            wrep_hi = big1.tile([MOUT - 128, F], f16, name="wrep_hi",
                                tag="wrep_hi")
            termSB = big1.tile([MD, F], f16, name="termSB", tag="termSB")
            termT = big1.tile([CD, w9], f16, name="termT", tag="termT")
            outSB = big1.tile([CD, w9], f32, name="outSB", tag="outSB")

            nc.sync.dma_start(out=pk[:, :], in_=pk_d)
            nc.scalar.dma_start(out=pk32[:, :], in_=b12_d)
            if stage <= 5:
                nc.gpsimd.memset(outSB[:, :], 0.0)

            for l, (c0, c1) in enumerate(lblk):
                w_l = w1t[:, l * CD:(l + 1) * CD]
                for s0 in range(c0, c1, FT):
                    s1 = min(s0 + FT, c1)
                    py = ps_t.tile([CD, FT], f32, name="py", tag="t")
                    nc.tensor.matmul(py[:, :s1 - s0], lhsT=w_l,
                                     rhs=xt[:, s0:s1], start=True, stop=True)
                    if l == 0:
                        nc.scalar.activation(ysb[:, s0:s1], py[:, :s1 - s0],
                                             AF.Identity, bias=sb_b1)
                    else:
                        nc.scalar.activation(ysb[:, s0:s1], py[:, :s1 - s0],
                                             AF.Copy)

            if stage <= 1:
                nc.sync.dma_start(out=out_d, in_=outSB)
                return

            for m in range(MD):
                eng = (nc.sync, nc.scalar, nc.gpsimd)[m % 3]
                eng.dma_start(out=x_d[m:m + 1, :],
                              in_=ysb[:, m * nloc:(m + 1) * nloc])
            nc.scalar.dma_start(out=mon_hi[N2:N2 + MD, :], in_=x_d)
            nc.gpsimd.dma_start(out=mon_hi[54:63, :], in_=x_d)
            nc.gpsimd.dma_start(out=mon_hi[63:64, :], in_=x_d[0:1, :])

            if stage <= 2:
                nc.sync.dma_start(out=out_d, in_=outSB)
                return

            for d in range(MD):
                q0, cnt = DOFF[d], MD - d
                nc.sync.dma_start(out=xpre_d[q0:q0 + cnt, :],
                                  in_=x_d[0:cnt, :])
                nc.gpsimd.dma_start(out=xsuf_d[q0:q0 + cnt, :],
                                     in_=x_d[d:MD, :])
            h2 = F // 2
            for c0, c1 in ((0, h2), (h2, F)):
                cs = slice(c0, c1)
                nc.sync.dma_start(out=xpre[:, cs], in_=xpre_d[:, c0:c1])
                nc.scalar.dma_start(out=xsuf[:, cs], in_=xsuf_d[:, c0:c1])
                t0 = 0
                for e in range(1, MD):
                    ln = N2 - DOFF[e]
                    eng = (nc.sync, nc.scalar)[e % 2]
                    eng.dma_start(out=xrep[t0:t0 + ln, cs],
                                  in_=xsuf_d[DOFF[e]:N2, c0:c1])
                    t0 += ln
                nc.vector.tensor_mul(mon_hi[0:N2, cs], xpre[:, cs],
                                     xsuf[:, cs])
                nc.vector.tensor_mul(mon_hi[64:64 + E0, cs],
                                     mon_hi[0:N2, cs], xsuf[:, cs])

            if stage <= 3:
                nc.sync.dma_start(out=out_d, in_=outSB)
                return

            if stage >= 5:
                h = F // 2
                nc.gpsimd.dma_start(out=wrep_lo[:, 0:h],
                                    in_=wrep_d[0:128, 0:h])
                nc.gpsimd.dma_start(out=wrep_lo[:, h:F],
                                    in_=wrep_d[0:128, h:F])
                nc.gpsimd.dma_start(out=wrep_hi[:, 0:h],
                                    in_=wrep_d[128:MOUT, 0:h])
                nc.gpsimd.dma_start(out=wrep_hi[:, h:F],
                                    in_=wrep_d[128:MOUT, h:F])

            for p in range(nft // 2):
                jp = slice(p * FT2, (p + 1) * FT2)
                ps1 = ps_r.tile([E1, FT2], f32, name="ps1", tag="r1")
                for h in range(2):
                    js = slice((2 * p + h) * FT, (2 * p + h + 1) * FT)
                    hs = slice(h * FT, (h + 1) * FT)
                    nc.tensor.matmul(ps1[:, hs], lhsT=sel1,
                                     rhs=mon_hi[0:N2, js],
                                     start=True, stop=True)
                nc.vector.tensor_mul(mon_lo[:, jp], xrep[:, jp], ps1[:])
                if stage <= 4:
                    continue
                dlo = ps_d.tile([128, FT2], f32, name="dlo", tag="dlo")
                pt = ps_t.tile([MD, FT2], f32, name="pt", tag="t")
                for h in range(2):
                    js = slice((2 * p + h) * FT, (2 * p + h + 1) * FT)
                    hs = slice(h * FT, (h + 1) * FT)
                    nc.tensor.matmul(dlo[:, hs], lhsT=uclo[:, 0:128],
                                     rhs=mon_lo[:, js], start=True, stop=False)
                    nc.tensor.matmul(dlo[:, hs], lhsT=uchi[:, 0:128],
                                     rhs=mon_hi[0:KHI, js],
                                     start=False, stop=True)
                dhi = ps_d.tile([MOUT - 128, FT2], f32, name="dhi",
                                tag="dhi")
                for h in range(2):
                    js = slice((2 * p + h) * FT, (2 * p + h + 1) * FT)
                    hs = slice(h * FT, (h + 1) * FT)
                    nc.tensor.matmul(dhi[:, hs], lhsT=uclo[:, 128:MOUT],
                                     rhs=mon_lo[:, js], start=True, stop=False)
                    nc.tensor.matmul(dhi[:, hs], lhsT=uchi[:, 128:MOUT],
                                     rhs=mon_hi[0:KHI, js],
                                     start=False, stop=True)
                dsl = work.tile([128, FT2], f16, name="dsl", tag="dsl")
                dsh = work.tile([MOUT - 128, FT2], f16, name="dsh", tag="dsh")
                nc.scalar.activation(dsl[:], dlo[:], AF.Copy)
                nc.scalar.activation(dsh[:], dhi[:], AF.Copy)
                dwl = work.tile([128, FT2], f16, name="dwl", tag="dwl")
                dwh = work.tile([MOUT - 128, FT2], f16, name="dwh", tag="dwh")
                nc.vector.tensor_mul(dwl[:], dsl[:], wrep_lo[:, jp])
                nc.vector.tensor_mul(dwh[:], dsh[:], wrep_hi[:, jp])
                for h in range(2):
                    hs = slice(h * FT, (h + 1) * FT)
                    nc.tensor.matmul(pt[:, hs], lhsT=glo, rhs=dwl[:, hs],
                                     start=True, stop=False)
                    nc.tensor.matmul(pt[:, hs], lhsT=ghi, rhs=dwh[:, hs],
                                     start=False, stop=True)
                nc.scalar.activation(termSB[:, jp], pt[:], AF.Copy)

            if stage <= 5:
                nc.sync.dma_start(out=out_d, in_=outSB)
                return

            for o in range(MD):
                dst = termT[:, o * nloc:(o + 1) * nloc]
                src = termSB[o:o + 1, :]
                if o % 2 == 0:
                    nc.sync.dma_start(out=dst, in_=src)
                else:
                    nc.gpsimd.dma_start(out=dst, in_=src)

            for l, (c0, c1) in enumerate(lblk):
                w_l = wct[:, l * CD:(l + 1) * CD]
                for s0 in range(c0, c1, FT):
                    s1 = min(s0 + FT, c1)
                    pf = ps_t.tile([CD, FT], f32, name="pf", tag="t")
                    nc.tensor.matmul(pf[:, :s1 - s0], lhsT=w_l,
                                     rhs=termT[:, s0:s1], start=True, stop=True)
                    if l == 0:
                        nc.scalar.activation(outSB[:, s0:s1], pf[:, :s1 - s0],
                                             AF.Identity, bias=sb_b2)
                    else:
                        nc.scalar.activation(outSB[:, s0:s1], pf[:, :s1 - s0],
                                             AF.Copy)
            nc.sync.dma_start(out=out_d, in_=outSB)

          if repeat > 1:
              with tc.For_i(0, repeat, 1):
                  _emit()
          else:
              _emit()

    return nc


def _get_program(nloc, repeat=1, stage=6):
    key = (nloc, repeat, stage)
    if key not in _PROGRAM:
        nc = _build_program(nloc, repeat, stage)
        nc.compile()
        _PROGRAM[key] = nc
    return _PROGRAM[key]


def make_in_maps(irreps_x, atomic_numbers, w_fc1, b_fc1, U3, W3, U2, W2, U1, W1,
                 w_lin, w_fc2, b_fc2, nloc=NLOC, ncores=NCORES):
    irreps_x = np.asarray(irreps_x, np.float32)
    a_n = np.asarray(atomic_numbers).astype(np.int64)
    U3c, U2c = _sym_compress(np.asarray(U3, np.float64),
                             np.asarray(U2, np.float64))
    UA, UB, G = _build_ucomb_g(U3c, U2c, np.asarray(U1, np.float32))
    sel1 = _build_sel1()
    w_comb = np.einsum('lde,lec->ldc', np.asarray(w_fc2, np.float32),
                       np.asarray(w_lin, np.float32))
    w1t = np.concatenate([np.asarray(w_fc1, np.float32)[l].T for l in range(3)],
                         axis=1)
    wct = np.concatenate([w_comb[l].T for l in range(3)], axis=1)
    w3g = np.asarray(W3, np.float32)[a_n]
    w2g = np.asarray(W2, np.float32)[a_n]
    w1g = np.asarray(W1, np.float32)[a_n]
    F = nloc * CD

    def put(buf, nm, arr, r0=0):
        o = PK_OFF[nm]
        arr = np.asarray(arr, np.float32).astype(np.float16)
        buf[r0:r0 + arr.shape[0], o:o + arr.shape[1]] = arr

    b12 = np.stack([np.asarray(b_fc1, np.float32),
                    np.asarray(b_fc2, np.float32)], axis=1).astype(np.float32)
    in_maps = []
    for core in range(ncores):
        s = slice(core * nloc, (core + 1) * nloc)
        parts = []
        for l in range(3):
            seg = irreps_x[s, l * l:(l + 1) * (l + 1), :]
            parts.append(seg.transpose(2, 1, 0).reshape(CD, -1))
        xtc = np.concatenate(parts, axis=1)
        pk = np.zeros((128, PK_BASE + MD * nloc), np.float16)
        put(pk, "uclo", UA)
        put(pk, "uchi", UB)
        put(pk, "glo", G[0:128])
        put(pk, "ghi", G[128:MOUT])
        put(pk, "w1t", w1t)
        put(pk, "wct", wct)
        put(pk, "sel1", sel1)
        pk[:CD, PK_BASE:PK_BASE + MD * nloc] = xtc.astype(np.float16)
        wg3 = w3g[s].transpose(1, 2, 0).reshape(P3D, F)
        wg2 = w2g[s].transpose(1, 2, 0).reshape(P2D, F)
        wg1 = w1g[s].transpose(1, 2, 0).reshape(P1D, F)
        wrep = np.concatenate([np.tile(wg3, (MD, 1)), np.tile(wg2, (MD, 1)),
                               np.tile(wg1, (MD, 1))], axis=0)
        in_maps.append({
            "pk": pk,
            "b12": b12,
            "wrep": wrep.astype(np.float16),
        })
    return in_maps


def unpack_out(o, nloc=NLOC):
    return np.ascontiguousarray(
        o.reshape(CD, MD, nloc).transpose(2, 1, 0)).astype(np.float32)


def kernel(**inputs):
    from concourse import bass_utils
    in_maps = make_in_maps(**inputs)
    nc = _get_program(NLOC)
    res = bass_utils.run_bass_kernel_spmd(nc, in_maps,
                                          core_ids=list(range(NCORES)))
    outs = [unpack_out(res.results[c]["out"]) for c in range(NCORES)]
    return np.concatenate(outs, axis=0).astype(np.float32)
